# revision 1
# baseline (speedup 1.0000x reference)
"""Trainium2 Bass kernel for nn_Block2DGRU: LN -> dw3x3 conv -> bidirectional
minGRU -> MLP, data-parallel over batch (32 samples -> 8 cores x 4).

Layout: everything on-device uses the transposed per-sample layout [d, L]
(channels on partitions, sequence on the free dim). The host transposes
x in/out. The minGRU log-space Heinsen scan is computed equivalently in
linear space with the DVE tensor_tensor_scan instruction
(h_t = a_t * h_{t-1} + b_t); direction 2 uses negative-stride APs.
"""
import numpy as np
import concourse.bacc as bacc
import concourse.tile as tile
import concourse.mybir as mybir
from concourse.bass_utils import run_bass_kernel_spmd

N_CORES = 8
NS = 4          # samples per core
DIM = 384
DI = 768        # minGRU inner dim
MLPD = 1536
L = 1024        # 32*32 flattened grid
GH = GW = 32
EPS = 1e-5
NC_D = DIM // 128    # 3 channel chunks
NC_J = (2 * DI) // 128  # 12 hg feature chunks
NC_H = DI // 128     # 6 hidden chunks
NC_M = MLPD // 128   # 12 mlp chunks

f32 = mybir.dt.float32
f32r = mybir.dt.float32r
f16 = mybir.dt.float16
Alu = mybir.AluOpType
Act = mybir.ActivationFunctionType

PADW = 34
PADN = PADW * PADW  # 1156


def build_nc(ns=NS, num_devices=N_CORES, ablate=(), ln_trivial=True):
    nc = bacc.Bacc("TRN2", target_bir_lowering=False, debug=False,
                   num_devices=num_devices)

    # ---- DRAM I/O ----
    xT_d = nc.dram_tensor("xT", [ns, DIM, L], f32r, kind="ExternalInput")
    g1w_d = nc.dram_tensor("g1w", [DIM, 2 * DI], f16, kind="ExternalInput")
    g2w_d = nc.dram_tensor("g2w", [DIM, 2 * DI], f16, kind="ExternalInput")
    g1o_d = nc.dram_tensor("g1o", [DI, DIM], f16, kind="ExternalInput")
    g2o_d = nc.dram_tensor("g2o", [DI, DIM], f16, kind="ExternalInput")
    p1w_d = nc.dram_tensor("p1w", [DIM, MLPD], f16, kind="ExternalInput")
    p2w_d = nc.dram_tensor("p2w", [MLPD, DIM], f16, kind="ExternalInput")
    cdiag_d = nc.dram_tensor("cdiag", [9, NC_D, 128, 128], f16,
                             kind="ExternalInput")
    pb1_d = nc.dram_tensor("pb1", [128, NC_M], f32, kind="ExternalInput")
    pb2_d = nc.dram_tensor("pb2", [128, NC_D], f32, kind="ExternalInput")
    dwcb_d = nc.dram_tensor("dwcb", [128, NC_D], f32, kind="ExternalInput")
    gm1_d = nc.dram_tensor("gm1", [128, NC_D], f32, kind="ExternalInput")
    bt1_d = nc.dram_tensor("bt1", [128, NC_D], f32, kind="ExternalInput")
    gm2_d = nc.dram_tensor("gm2", [128, NC_D], f32, kind="ExternalInput")
    bt2_d = nc.dram_tensor("bt2", [128, NC_D], f32, kind="ExternalInput")
    y_d = nc.dram_tensor("y", [ns, DIM, L], f32, kind="ExternalOutput")

    with tile.TileContext(nc) as tc:
        with tc.tile_pool(name="wp", bufs=1) as wp, \
             tc.tile_pool(name="sb", bufs=1) as sb, \
             tc.tile_pool(name="ps", bufs=2, space="PSUM") as ps:

            # ---- persistent weights in SBUF ----
            ones32 = sb.tile([128, 512], f32, tag="sq", name="ones32", bufs=2)
            nc.vector.memset(ones32[:, :128], 1.0)
            ones = wp.tile([128, 128], f32r, tag="ones", name="ones")
            nc.scalar.copy(ones[:], ones32[:, :128])
            epsc = wp.tile([128, 1], f32, tag="epsc", name="epsc")
            nc.vector.memset(epsc[:], EPS)
            g1w_t = [wp.tile([128, 2 * DI], f16, tag=f"g1w{k}", name=f"g1w{k}") for k in range(NC_D)]
            g2w_t = [wp.tile([128, 2 * DI], f16, tag=f"g2w{k}", name=f"g2w{k}") for k in range(NC_D)]
            g1o_t = [wp.tile([128, DIM], f16, tag=f"g1o{j}", name=f"g1o{j}") for j in range(NC_H)]
            g2o_t = [wp.tile([128, DIM], f16, tag=f"g2o{j}", name=f"g2o{j}") for j in range(NC_H)]
            p1w_t = [wp.tile([128, MLPD], f16, tag=f"p1w{k}", name=f"p1w{k}") for k in range(NC_D)]
            p2w_t = [wp.tile([128, DIM], f16, tag=f"p2w{j}", name=f"p2w{j}") for j in range(NC_M)]
            cd_t = [[wp.tile([128, 128], f16, tag=f"cd{t}_{c}", name=f"cd{t}_{c}")
                     for c in range(NC_D)] for t in range(9)]
            pb1_t = wp.tile([128, NC_M], f32, tag="pb1", name="pb1")
            pb2_t = wp.tile([128, NC_D], f32, tag="pb2", name="pb2")
            dwcb_t = wp.tile([128, NC_D], f32, tag="dwcb", name="dwcb")
            gm1_t = wp.tile([128, NC_D], f32, tag="gm1", name="gm1")
            bt1_t = wp.tile([128, NC_D], f32, tag="bt1", name="bt1")
            gm2_t = wp.tile([128, NC_D], f32, tag="gm2", name="gm2")
            bt2_t = wp.tile([128, NC_D], f32, tag="bt2", name="bt2")

            for k in range(NC_D):
                nc.sync.dma_start(g1w_t[k][:], g1w_d.ap()[k * 128:(k + 1) * 128, :])
                nc.sync.dma_start(g2w_t[k][:], g2w_d.ap()[k * 128:(k + 1) * 128, :])
                nc.sync.dma_start(p1w_t[k][:], p1w_d.ap()[k * 128:(k + 1) * 128, :])
            for j in range(NC_H):
                nc.sync.dma_start(g1o_t[j][:], g1o_d.ap()[j * 128:(j + 1) * 128, :])
                nc.sync.dma_start(g2o_t[j][:], g2o_d.ap()[j * 128:(j + 1) * 128, :])
            for j in range(NC_M):
                nc.sync.dma_start(p2w_t[j][:], p2w_d.ap()[j * 128:(j + 1) * 128, :])
            for t in range(9):
                for c in range(NC_D):
                    nc.sync.dma_start(cd_t[t][c][:], cdiag_d.ap()[t, c])
            for name, tl, dr in [("pb1", pb1_t, pb1_d), ("pb2", pb2_t, pb2_d),
                                 ("dwcb", dwcb_t, dwcb_d), ("gm1", gm1_t, gm1_d),
                                 ("bt1", bt1_t, bt1_d), ("gm2", gm2_t, gm2_d),
                                 ("bt2", bt2_t, bt2_d)]:
                nc.sync.dma_start(tl[:], dr.ap()[:])

            def layer_norm_stats_ablated(src_tiles, tag):
                tm = [sb.tile([128, L], f32, tag=f"tm{c}", name=f"tma{c}_{tag}")
                      for c in range(NC_D)]
                for c in range(NC_D):
                    nc.vector.memset(tm[c][:], 0.5)
                rstd = sb.tile([128, L], f32, tag="rstd", name=f"rstd_{tag}")
                nc.vector.memset(rstd[:], 1.0)
                return tm, rstd

            def layer_norm_stats(src_tiles, tag):
                if "ln" in ablate:
                    return layer_norm_stats_ablated(src_tiles, tag)
                """src_tiles: 3 x [128,1024] f32r. Returns (mu, rstd) SBUF
                f32 [128,1024] broadcast tiles."""
                m2 = sb.tile([128, L], f32, tag="m2", name=f"m2_{tag}", bufs=2)
                v = sb.tile([128, L], f32, tag="v", name=f"v_{tag}")
                tm = [sb.tile([128, L], f32, tag=f"tm{c}", name=f"tm{c}_{tag}")
                      for c in range(NC_D)]
                for h in range(2):
                    sl = slice(h * 512, (h + 1) * 512)
                    Sh = ps.tile([128, 512], f32, tag="bank",
                                 name=f"lnS{h}", bufs=8)
                    for c in range(NC_D):
                        nc.tensor.matmul(Sh[:], ones[:], src_tiles[c][:, sl],
                                         start=(c == 0), stop=(c == NC_D - 1))
                    SSh = ps.tile([128, 512], f32, tag="bank",
                                  name=f"lnSS{h}", bufs=8)
                    for c in range(NC_D):
                        sqc = sb.tile([128, 512], f32r, tag="sq",
                                      name=f"sq{c}{h}_{tag}", bufs=2)
                        nc.scalar.activation(
                            sqc[:], src_tiles[c][:, sl].bitcast(f32),
                            Act.Square)
                        nc.tensor.matmul(SSh[:], ones[:], sqc[:],
                                         start=(c == 0), stop=(c == NC_D - 1))
                    nc.scalar.activation(m2[:, sl], Sh[:], Act.Square,
                                         scale=1.0 / DIM)
                    # t = x - mu  computed as (S * -1/DIM) + x
                    for c in range(NC_D):
                        nc.vector.scalar_tensor_tensor(
                            tm[c][:, sl], Sh[:], -1.0 / DIM,
                            src_tiles[c][:, sl].bitcast(f32),
                            op0=Alu.mult, op1=Alu.add)
                    nc.vector.scalar_tensor_tensor(
                        v[:, sl], SSh[:], 1.0 / DIM, m2[:, sl],
                        op0=Alu.mult, op1=Alu.subtract)
                sd = sb.tile([128, L], f32, tag="m2", name=f"sd_{tag}", bufs=2)
                nc.scalar.activation(sd[:], v[:], Act.Sqrt, bias=epsc[:])
                rstd = sb.tile([128, L], f32, tag="rstd", name=f"rstd_{tag}")
                nc.vector.reciprocal(rstd[:], sd[:])
                return tm, rstd

            for s in range(ns):
                # ---- load sample (transposed [384, 1024]) ----
                xt = [sb.tile([128, L], f32r, tag=f"xt{c}", name=f"xt{c}", bufs=2) for c in range(NC_D)]
                for c in range(NC_D):
                    nc.sync.dma_start(xt[c][:],
                                      xT_d.ap()[s, c * 128:(c + 1) * 128, :])

                # ---- LN1 + padded apply ----
                tm1, rstd1 = layer_norm_stats(xt, "1")
                xnp = [sb.tile([128, PADN], f16, tag=f"xnp{c}", name=f"xnp{c}")
                       for c in range(NC_D)]
                for c in range(NC_D):
                    nc.vector.memset(xnp[c][:], 0.0)
                    t1 = tm1[c]
                    xnp3 = xnp[c][:].rearrange("p (a b) -> p a b", a=PADW)
                    t13 = t1[:].rearrange("p (a b) -> p a b", a=GH)
                    r13 = rstd1[:].rearrange("p (a b) -> p a b", a=GH)
                    if ln_trivial:
                        nc.vector.tensor_tensor(
                            xnp3[:, 1:GH + 1, 1:GW + 1], t13[:, :, :],
                            r13[:, :, :], op=Alu.mult)
                    else:
                        nc.vector.scalar_tensor_tensor(
                            xnp3[:, 1:GH + 1, 1:GW + 1], t13[:, :, :],
                            gm1_t[:, c:c + 1], r13[:, :, :],
                            op0=Alu.mult, op1=Alu.mult)
                        nc.vector.tensor_scalar(
                            xnp3[:, 1:GH + 1, 1:GW + 1],
                            xnp3[:, 1:GH + 1, 1:GW + 1],
                            bt1_t[:, c:c + 1], None, op0=Alu.add)

                # ---- depthwise 3x3 conv on PE (9 diag matmuls / chunk) ----
                xs = [sb.tile([128, L], f16, tag=f"xs{c}", name=f"xs{c}") for c in range(NC_D)]
                if "conv" in ablate:
                    for c in range(NC_D):
                        nc.vector.memset(xs[c][:], 0.01)
                for c in range(NC_D if "conv" not in ablate else 0):
                    xp3 = xnp[c][:].rearrange("p (a b) -> p a b", a=PADW)
                    for h in range(2):
                        cv = ps.tile([128, 512], f32, tag="bank",
                                     name=f"cv{c}{h}", bufs=8)
                        for t in range(9):
                            ky, kx = divmod(t, 3)
                            rhs = xp3[:, ky + 16 * h: ky + 16 * h + 16,
                                      kx: kx + GW]
                            nc.tensor.matmul(cv[:], cd_t[t][c][:], rhs,
                                             start=(t == 0), stop=(t == 8))
                        sl = slice(h * 512, (h + 1) * 512)
                        nc.scalar.activation(xs[c][:, sl], cv[:],
                                             Act.Identity,
                                             bias=dwcb_t[:, c:c + 1])

                # ---- bidirectional minGRU ----
                y0 = None
                for d in range(2):
                    gw_t = g1w_t if d == 0 else g2w_t
                    go_t = g1o_t if d == 0 else g2o_t
                    hh = [sb.tile([128, L], f16, tag=f"hh{j}", name=f"hh{j}")
                          for j in range(NC_H)]
                    for j in range(NC_H):
                        Hp = [ps.tile([128, 512], f32, tag="bank",
                                      name=f"Hp{h}", bufs=8) for h in range(2)]
                        Gp = [ps.tile([128, 512], f32, tag="bank",
                                      name=f"Gp{h}", bufs=8) for h in range(2)]
                        for k in range(NC_D):
                            wj = gw_t[k][:, j * 128:(j + 1) * 128]
                            for h in range(2):
                                sl = slice(h * 512, (h + 1) * 512)
                                nc.tensor.matmul(
                                    Hp[h][:], wj, xs[k][:, sl],
                                    start=(k == 0), stop=(k == NC_D - 1))
                        for k in range(NC_D):
                            wj = gw_t[k][:, (j + NC_H) * 128:
                                        (j + NC_H + 1) * 128]
                            for h in range(2):
                                sl = slice(h * 512, (h + 1) * 512)
                                nc.tensor.matmul(
                                    Gp[h][:], wj, xs[k][:, sl],
                                    start=(k == 0), stop=(k == NC_D - 1))
                        if "ew" in ablate:
                            nc.vector.memset(hh[j][:], 0.01)
                            continue
                        z = sb.tile([128, L], f16, tag="z", name="z", bufs=2)
                        sh = sb.tile([128, L], f16, tag="sh", name="sh", bufs=2)
                        rh = sb.tile([128, L], f16, tag="rh", name="rh", bufs=2)
                        for h in range(2):
                            sl = slice(h * 512, (h + 1) * 512)
                            nc.scalar.activation(z[:, sl], Gp[h][:],
                                                 Act.Sigmoid)
                            nc.scalar.activation(sh[:, sl], Hp[h][:],
                                                 Act.Sigmoid)
                            nc.scalar.activation(rh[:, sl], Hp[h][:],
                                                 Act.Relu)
                        a = sb.tile([128, L], f16, tag="a", name="a")
                        nc.vector.tensor_scalar(a[:], z[:], -1.0, 1.0,
                                                op0=Alu.mult, op1=Alu.add)
                        sm = sb.tile([128, L], f16, tag="sm", name="sm")
                        nc.vector.tensor_scalar(sm[:], sh[:], 0.5, None,
                                                op0=Alu.min)
                        g = sb.tile([128, L], f16, tag="g", name="g")
                        nc.vector.tensor_tensor(g[:], rh[:], sm[:], op=Alu.add)
                        b = sb.tile([128, L], f16, tag="b", name="b")
                        nc.vector.tensor_tensor(b[:], z[:], g[:], op=Alu.mult)
                        if "scan" in ablate:
                            nc.vector.tensor_copy(hh[j][:], b[:])
                        elif d == 0:
                            nc.vector.tensor_tensor_scan(
                                hh[j][:], a[:], b[:], 0.0,
                                op0=Alu.mult, op1=Alu.add)
                        else:
                            nc.vector.tensor_tensor_scan(
                                hh[j][:, ::-1], a[:, ::-1], b[:, ::-1], 0.0,
                                op0=Alu.mult, op1=Alu.add)
                    # out-projection: x_d = h @ w_out -> stream per (m, half)
                    if d == 0:
                        y0 = [sb.tile([128, L], f32, tag=f"y0_{c}", name=f"y0_{c}")
                              for c in range(NC_D)]
                    else:
                        yt = [sb.tile([128, L], f32r, tag=f"yt{c}", name=f"yt{c}")
                              for c in range(NC_D)]
                    for m in range(NC_D):
                        for h in range(2):
                            q = ps.tile([128, 512], f32, tag="bank",
                                        name=f"xq{m}_{h}", bufs=8)
                            sl = slice(h * 512, (h + 1) * 512)
                            for j in range(NC_H):
                                nc.tensor.matmul(
                                    q[:], go_t[j][:, m * 128:(m + 1) * 128],
                                    hh[j][:, sl],
                                    start=(j == 0), stop=(j == NC_H - 1))
                            if d == 0:
                                nc.vector.tensor_tensor(
                                    y0[m][:, sl], q[:],
                                    xt[m][:, sl].bitcast(f32), op=Alu.add)
                            else:
                                nc.vector.tensor_tensor(
                                    yt[m][:, sl], q[:],
                                    y0[m][:, sl], op=Alu.add)

                # ---- LN2 + MLP ----
                tm2, rstd2 = layer_norm_stats(yt, "2")
                yn = [sb.tile([128, L], f16, tag=f"yn{c}", name=f"yn{c}") for c in range(NC_D)]
                for c in range(NC_D):
                    t1 = tm2[c]
                    if ln_trivial:
                        nc.vector.tensor_tensor(yn[c][:], t1[:], rstd2[:],
                                                op=Alu.mult)
                    else:
                        nc.vector.scalar_tensor_tensor(
                            yn[c][:], t1[:], gm2_t[:, c:c + 1], rstd2[:],
                            op0=Alu.mult, op1=Alu.mult)
                        nc.vector.tensor_scalar(
                            yn[c][:], yn[c][:], bt2_t[:, c:c + 1], None,
                            op0=Alu.add)
                yh = [sb.tile([128, L], f16, tag=f"yh{j}", name=f"yh{j}")
                      for j in range(NC_M)]
                for j in range(NC_M):
                    pph = [ps.tile([128, 512], f32, tag="bank",
                                   name=f"pp{h}", bufs=8) for h in range(2)]
                    for k in range(NC_D):
                        wj = p1w_t[k][:, j * 128:(j + 1) * 128]
                        for h in range(2):
                            sl = slice(h * 512, (h + 1) * 512)
                            nc.tensor.matmul(
                                pph[h][:], wj, yn[k][:, sl],
                                start=(k == 0), stop=(k == NC_D - 1))
                    for h in range(2):
                        sl = slice(h * 512, (h + 1) * 512)
                        nc.scalar.activation(yh[j][:, sl], pph[h][:], Act.Gelu,
                                             bias=pb1_t[:, j:j + 1])
                yo = [sb.tile([128, L], f32, tag=f"y0_{c}", name=f"yo{c}") for c in range(NC_D)]
                for m in range(NC_D):
                    for h in range(2):
                        sl = slice(h * 512, (h + 1) * 512)
                        q = ps.tile([128, 512], f32, tag="bank",
                                    name=f"oq{m}_{h}", bufs=8)
                        for j in range(NC_M):
                            nc.tensor.matmul(
                                q[:], p2w_t[j][:, m * 128:(m + 1) * 128],
                                yh[j][:, sl],
                                start=(j == 0), stop=(j == NC_M - 1))
                        nc.vector.scalar_tensor_tensor(
                            yo[m][:, sl], q[:], pb2_t[:, m:m + 1],
                            yt[m][:, sl].bitcast(f32),
                            op0=Alu.add, op1=Alu.add)
                for c in range(NC_D):
                    nc.sync.dma_start(y_d.ap()[s, c * 128:(c + 1) * 128, :],
                                      yo[c][:])

    nc.compile()
    return nc


_NC_CACHE = {}


def _get_nc(ns=NS, num_devices=N_CORES, ln_trivial=True):
    key = (ns, num_devices, ln_trivial)
    if key not in _NC_CACHE:
        _NC_CACHE[key] = build_nc(ns, num_devices, ln_trivial=ln_trivial)
    return _NC_CACHE[key]


def make_weight_maps(gamma1, beta1, dwc_w, dwc_b, gru1_w, gru1_out,
                     gru2_w, gru2_out, gamma2, beta2, p1_w, p1_b, p2_w, p2_b):
    f = np.float32
    dwc = np.asarray(dwc_w, f).reshape(DIM, 9)
    cdiag = np.zeros((9, NC_D, 128, 128), np.float16)
    for t in range(9):
        for c in range(NC_D):
            np.einsum('ii->i', cdiag[t, c])[:] = dwc[c * 128:(c + 1) * 128, t]
    col = lambda v: np.ascontiguousarray(
        np.asarray(v, f).reshape(NC_D if len(np.asarray(v).reshape(-1)) == DIM
                                 else NC_M, 128).T)
    return dict(
        g1w=np.asarray(gru1_w, f).astype(np.float16),
        g2w=np.asarray(gru2_w, f).astype(np.float16),
        g1o=np.asarray(gru1_out, f).astype(np.float16),
        g2o=np.asarray(gru2_out, f).astype(np.float16),
        p1w=np.asarray(p1_w, f).astype(np.float16),
        p2w=np.asarray(p2_w, f).astype(np.float16),
        cdiag=cdiag,
        pb1=col(p1_b), pb2=col(p2_b), dwcb=col(dwc_b),
        gm1=col(gamma1), bt1=col(beta1), gm2=col(gamma2), bt2=col(beta2),
    )


def kernel(x, gamma1, beta1, dwc_w, dwc_b, gru1_w, gru1_out, gru2_w, gru2_out,
           gamma2, beta2, p1_w, p1_b, p2_w, p2_b, h, w):
    x = np.asarray(x, np.float32)
    n = x.shape[0]
    ln_trivial = (np.allclose(np.asarray(gamma1), 1.0)
                  and np.allclose(np.asarray(beta1), 0.0)
                  and np.allclose(np.asarray(gamma2), 1.0)
                  and np.allclose(np.asarray(beta2), 0.0))
    nc = _get_nc(ln_trivial=ln_trivial)
    xT = np.ascontiguousarray(x.transpose(0, 2, 1))  # [32, 384, 1024]
    wmap = make_weight_maps(gamma1, beta1, dwc_w, dwc_b, gru1_w, gru1_out,
                            gru2_w, gru2_out, gamma2, beta2, p1_w, p1_b,
                            p2_w, p2_b)
    in_maps = []
    for i in range(N_CORES):
        m = dict(wmap)
        m["xT"] = xT[i * NS:(i + 1) * NS]
        in_maps.append(m)
    res = run_bass_kernel_spmd(nc, in_maps, list(range(N_CORES)))
    yT = np.concatenate([res.results[i]["y"] for i in range(N_CORES)], axis=0)
    return np.ascontiguousarray(yT.transpose(0, 2, 1)).astype(np.float32)



# revision 4
# speedup vs baseline: 2.1185x; 2.1185x over previous
"""Trainium2 Bass kernel for nn_Block2DGRU: LN -> dw3x3 conv -> bidirectional
minGRU -> MLP, data-parallel over batch (32 samples -> 8 cores x 4).

Layout: per-sample transposed [d, L] (channels on partitions, sequence on
free dim). minGRU scan in linear space via DVE tensor_tensor_scan
(h_t = a_t*h_{t-1} + b_t); direction 2 via negative-stride APs.

v2: 2-bank PSUM tiles (FD=1024 ACT/DVE ops), weight-stationary-reuse matmul
order, separate LN1/LN2 temp tags + double-buffered GRU temps for
cross-sample overlap, f16 input/residual tiles.
"""
import numpy as np
import concourse.bacc as bacc
import concourse.tile as tile
import concourse.mybir as mybir
from concourse.bass_utils import run_bass_kernel_spmd

N_CORES = 8
NS = 4          # samples per core
DIM = 384
DI = 768        # minGRU inner dim
MLPD = 1536
L = 1024        # 32*32 flattened grid
GH = GW = 32
EPS = 1e-5
NC_D = DIM // 128    # 3 channel chunks
NC_H = DI // 128     # 6 hidden chunks
NC_M = MLPD // 128   # 12 mlp chunks

f32 = mybir.dt.float32
f32r = mybir.dt.float32r
f16 = mybir.dt.float16
Alu = mybir.AluOpType
Act = mybir.ActivationFunctionType

PADW = 34
PADN = PADW * PADW  # 1156
H0 = slice(0, 512)
H1 = slice(512, 1024)
HS = (H0, H1)


def build_nc(ns=NS, num_devices=N_CORES, ablate=(), ln_trivial=True):
    nc = bacc.Bacc("TRN2", target_bir_lowering=False, debug=False,
                   num_devices=num_devices)

    # ---- DRAM I/O ----
    xT_d = nc.dram_tensor("xT", [ns, DIM, L], f16, kind="ExternalInput")
    g1w_d = nc.dram_tensor("g1w", [DIM, 2 * DI], f16, kind="ExternalInput")
    g2w_d = nc.dram_tensor("g2w", [DIM, 2 * DI], f16, kind="ExternalInput")
    g1o_d = nc.dram_tensor("g1o", [DI, DIM], f16, kind="ExternalInput")
    g2o_d = nc.dram_tensor("g2o", [DI, DIM], f16, kind="ExternalInput")
    p1w_d = nc.dram_tensor("p1w", [DIM, MLPD], f16, kind="ExternalInput")
    p2w_d = nc.dram_tensor("p2w", [MLPD, DIM], f16, kind="ExternalInput")
    cdiag_d = nc.dram_tensor("cdiag", [9, NC_D, 128, 128], f16,
                             kind="ExternalInput")
    pb1_d = nc.dram_tensor("pb1", [128, NC_M], f32, kind="ExternalInput")
    pb2_d = nc.dram_tensor("pb2", [128, NC_D], f32, kind="ExternalInput")
    dwcb_d = nc.dram_tensor("dwcb", [128, NC_D], f32, kind="ExternalInput")
    gm1_d = nc.dram_tensor("gm1", [128, NC_D], f32, kind="ExternalInput")
    bt1_d = nc.dram_tensor("bt1", [128, NC_D], f32, kind="ExternalInput")
    gm2_d = nc.dram_tensor("gm2", [128, NC_D], f32, kind="ExternalInput")
    bt2_d = nc.dram_tensor("bt2", [128, NC_D], f32, kind="ExternalInput")
    y_d = nc.dram_tensor("y", [ns, DIM, L], f16, kind="ExternalOutput")

    with tile.TileContext(nc) as tc:
        with nc.allow_low_precision(reason="f16 intermediates within 2e-2 tol"), \
             tc.tile_pool(name="wp", bufs=1) as wp, \
             tc.tile_pool(name="sb", bufs=1) as sb, \
             tc.tile_pool(name="ps", bufs=4, space="PSUM") as ps:

            # ---- persistent weights in SBUF ----
            onesf = wp.tile([128, 128], f16, tag="onesf", name="onesf")
            nc.vector.memset(onesf[:], 1.0)
            epsc = wp.tile([128, 1], f32, tag="epsc", name="epsc")
            nc.vector.memset(epsc[:], EPS)
            g1w_t = [wp.tile([128, 2 * DI], f16, tag=f"g1w{k}", name=f"g1w{k}") for k in range(NC_D)]
            g2w_t = [wp.tile([128, 2 * DI], f16, tag=f"g2w{k}", name=f"g2w{k}") for k in range(NC_D)]
            g1o_t = [wp.tile([128, DIM], f16, tag=f"g1o{j}", name=f"g1o{j}") for j in range(NC_H)]
            g2o_t = [wp.tile([128, DIM], f16, tag=f"g2o{j}", name=f"g2o{j}") for j in range(NC_H)]
            p1w_t = [wp.tile([128, MLPD], f16, tag=f"p1w{k}", name=f"p1w{k}") for k in range(NC_D)]
            p2w_t = [wp.tile([128, DIM], f16, tag=f"p2w{j}", name=f"p2w{j}") for j in range(NC_M)]
            cd_t = [[wp.tile([128, 128], f16, tag=f"cd{t}_{c}", name=f"cd{t}_{c}")
                     for c in range(NC_D)] for t in range(9)]
            pb1_t = wp.tile([128, NC_M], f32, tag="pb1", name="pb1")
            pb2_t = wp.tile([128, NC_D], f32, tag="pb2", name="pb2")
            dwcb_t = wp.tile([128, NC_D], f32, tag="dwcb", name="dwcb")
            gm1_t = wp.tile([128, NC_D], f32, tag="gm1", name="gm1")
            bt1_t = wp.tile([128, NC_D], f32, tag="bt1", name="bt1")
            gm2_t = wp.tile([128, NC_D], f32, tag="gm2", name="gm2")
            bt2_t = wp.tile([128, NC_D], f32, tag="bt2", name="bt2")

            for k in range(NC_D):
                nc.sync.dma_start(g1w_t[k][:], g1w_d.ap()[k * 128:(k + 1) * 128, :])
                nc.sync.dma_start(g2w_t[k][:], g2w_d.ap()[k * 128:(k + 1) * 128, :])
                nc.sync.dma_start(p1w_t[k][:], p1w_d.ap()[k * 128:(k + 1) * 128, :])
            for j in range(NC_H):
                nc.sync.dma_start(g1o_t[j][:], g1o_d.ap()[j * 128:(j + 1) * 128, :])
                nc.sync.dma_start(g2o_t[j][:], g2o_d.ap()[j * 128:(j + 1) * 128, :])
            for j in range(NC_M):
                nc.sync.dma_start(p2w_t[j][:], p2w_d.ap()[j * 128:(j + 1) * 128, :])
            for t in range(9):
                for c in range(NC_D):
                    nc.sync.dma_start(cd_t[t][c][:], cdiag_d.ap()[t, c])
            for name, tl, dr in [("pb1", pb1_t, pb1_d), ("pb2", pb2_t, pb2_d),
                                 ("dwcb", dwcb_t, dwcb_d), ("gm1", gm1_t, gm1_d),
                                 ("bt1", bt1_t, bt1_d), ("gm2", gm2_t, gm2_d),
                                 ("bt2", bt2_t, bt2_d)]:
                nc.sync.dma_start(tl[:], dr.ap()[:])

            def layer_norm_stats(src_tiles, tag):
                """src_tiles: 3 x [128,1024] f16. Returns (tm, rstd): f16
                (x-mu) chunk tiles and f16 [128,1024] rstd tile."""
                if "ln" in ablate:
                    tm = [sb.tile([128, L], f16, tag=f"tm{c}{tag}",
                                  name=f"tma{c}_{tag}") for c in range(NC_D)]
                    for c in range(NC_D):
                        nc.vector.memset(tm[c][:], 0.5)
                    rstd = sb.tile([128, L], f16, tag=f"rstd{tag}",
                                   name=f"rstd_{tag}")
                    nc.vector.memset(rstd[:], 1.0)
                    return tm, rstd
                Sh = ps.tile([128, L], f32, tag="bk", name=f"lnS_{tag}")
                SSh = ps.tile([128, L], f32, tag="bk", name=f"lnSS_{tag}")
                for c in range(NC_D):
                    for h in HS:
                        nc.tensor.matmul(Sh[:, h], onesf[:], src_tiles[c][:, h],
                                         start=(c == 0), stop=(c == NC_D - 1))
                sq = sb.tile([128, L], f16, tag=f"sq{tag}", name=f"sq_{tag}",
                             bufs=2)
                for c in range(NC_D):
                    nc.scalar.activation(sq[:], src_tiles[c][:], Act.Square)
                    for h in HS:
                        nc.tensor.matmul(SSh[:, h], onesf[:], sq[:, h],
                                         start=(c == 0), stop=(c == NC_D - 1))
                m2 = sb.tile([128, L], f32, tag=f"m2{tag}", name=f"m2_{tag}",
                             bufs=2)
                nc.scalar.activation(m2[:], Sh[:], Act.Square, scale=1.0 / DIM)
                tm = [sb.tile([128, L], f16, tag=f"tm{c}{tag}",
                              name=f"tm{c}_{tag}") for c in range(NC_D)]
                for c in range(NC_D):
                    nc.vector.scalar_tensor_tensor(
                        tm[c][:], Sh[:], -1.0 / DIM, src_tiles[c][:],
                        op0=Alu.mult, op1=Alu.add)
                v = sb.tile([128, L], f32, tag=f"v{tag}", name=f"v_{tag}")
                nc.vector.scalar_tensor_tensor(
                    v[:], SSh[:], 1.0 / DIM, m2[:],
                    op0=Alu.mult, op1=Alu.subtract)
                sd = sb.tile([128, L], f32, tag=f"m2{tag}", name=f"sd_{tag}",
                             bufs=2)
                nc.scalar.activation(sd[:], v[:], Act.Sqrt, bias=epsc[:])
                rstd = sb.tile([128, L], f16, tag=f"rstd{tag}",
                               name=f"rstd_{tag}")
                nc.vector.reciprocal(rstd[:], sd[:])
                return tm, rstd

            for s in range(ns):
                # ---- load sample (transposed [384, 1024] f16) ----
                xt = [sb.tile([128, L], f16, tag=f"xt{c}", name=f"xt{c}",
                              bufs=2) for c in range(NC_D)]
                for c in range(NC_D):
                    nc.sync.dma_start(xt[c][:],
                                      xT_d.ap()[s, c * 128:(c + 1) * 128, :])

                # ---- LN1 + padded apply ----
                tm1, rstd1 = layer_norm_stats(xt, "a")
                xnp = [sb.tile([128, PADN], f16, tag=f"xnp{c}", name=f"xnp{c}")
                       for c in range(NC_D)]
                for c in range(NC_D):
                    nc.vector.memset(xnp[c][:], 0.0)
                    xnp3 = xnp[c][:].rearrange("p (a b) -> p a b", a=PADW)
                    t13 = tm1[c][:].rearrange("p (a b) -> p a b", a=GH)
                    r13 = rstd1[:].rearrange("p (a b) -> p a b", a=GH)
                    if ln_trivial:
                        nc.vector.tensor_tensor(
                            xnp3[:, 1:GH + 1, 1:GW + 1], t13[:, :, :],
                            r13[:, :, :], op=Alu.mult)
                    else:
                        nc.vector.scalar_tensor_tensor(
                            xnp3[:, 1:GH + 1, 1:GW + 1], t13[:, :, :],
                            gm1_t[:, c:c + 1], r13[:, :, :],
                            op0=Alu.mult, op1=Alu.mult)
                        nc.vector.tensor_scalar(
                            xnp3[:, 1:GH + 1, 1:GW + 1],
                            xnp3[:, 1:GH + 1, 1:GW + 1],
                            bt1_t[:, c:c + 1], None, op0=Alu.add)

                # ---- depthwise 3x3 conv on PE (9 diag matmuls / chunk) ----
                xs = [sb.tile([128, L], f16, tag=f"xs{c}", name=f"xs{c}")
                      for c in range(NC_D)]
                if "conv" in ablate:
                    for c in range(NC_D):
                        nc.vector.memset(xs[c][:], 0.01)
                for c in range(NC_D if "conv" not in ablate else 0):
                    xp3 = xnp[c][:].rearrange("p (a b) -> p a b", a=PADW)
                    cv = ps.tile([128, L], f32, tag="bk", name=f"cv{c}")
                    for t in range(9):
                        ky, kx = divmod(t, 3)
                        for h in range(2):
                            rhs = xp3[:, ky + 16 * h: ky + 16 * h + 16,
                                      kx: kx + GW]
                            nc.tensor.matmul(cv[:, HS[h]], cd_t[t][c][:], rhs,
                                             start=(t == 0), stop=(t == 8))
                    nc.scalar.activation(xs[c][:], cv[:], Act.Identity,
                                         bias=dwcb_t[:, c:c + 1])

                # ---- bidirectional minGRU ----
                y0 = None
                for d in range(2):
                    gw_t = g1w_t if d == 0 else g2w_t
                    go_t = g1o_t if d == 0 else g2o_t
                    hh = [sb.tile([128, L], f16, tag=f"hh{j}", name=f"hh{j}")
                          for j in range(NC_H)]
                    for j in range(NC_H):
                        Hp = ps.tile([128, L], f32, tag="bk", name=f"Hp{j}")
                        Gp = ps.tile([128, L], f32, tag="bk", name=f"Gp{j}")
                        for k in range(NC_D):
                            wj = gw_t[k][:, j * 128:(j + 1) * 128]
                            for h in HS:
                                nc.tensor.matmul(
                                    Hp[:, h], wj, xs[k][:, h],
                                    start=(k == 0), stop=(k == NC_D - 1))
                        for k in range(NC_D):
                            wj = gw_t[k][:, (j + NC_H) * 128:
                                        (j + NC_H + 1) * 128]
                            for h in HS:
                                nc.tensor.matmul(
                                    Gp[:, h], wj, xs[k][:, h],
                                    start=(k == 0), stop=(k == NC_D - 1))
                        if "ew" in ablate:
                            nc.vector.memset(hh[j][:], 0.01)
                            continue
                        z = sb.tile([128, L], f16, tag="z", name="z", bufs=2)
                        sh = sb.tile([128, L], f16, tag="sh", name="sh", bufs=2)
                        rh = sb.tile([128, L], f16, tag="rh", name="rh", bufs=2)
                        nc.scalar.activation(z[:], Gp[:], Act.Sigmoid)
                        nc.scalar.activation(sh[:], Hp[:], Act.Sigmoid)
                        nc.scalar.activation(rh[:], Hp[:], Act.Relu)
                        g = sb.tile([128, L], f16, tag="g", name="g", bufs=2)
                        nc.vector.scalar_tensor_tensor(
                            g[:], sh[:], 0.5, rh[:], op0=Alu.min, op1=Alu.add)
                        b = sb.tile([128, L], f16, tag="b", name="b", bufs=2)
                        nc.vector.tensor_tensor(b[:], z[:], g[:], op=Alu.mult)
                        a = sb.tile([128, L], f16, tag="a", name="a", bufs=2)
                        nc.vector.tensor_scalar(a[:], z[:], -1.0, 1.0,
                                                op0=Alu.mult, op1=Alu.add)
                        if "scan" in ablate:
                            nc.vector.tensor_copy(hh[j][:], b[:])
                        elif d == 0:
                            nc.vector.tensor_tensor_scan(
                                hh[j][:], a[:], b[:], 0.0,
                                op0=Alu.mult, op1=Alu.add)
                        else:
                            nc.vector.tensor_tensor_scan(
                                hh[j][:, ::-1], a[:, ::-1], b[:, ::-1], 0.0,
                                op0=Alu.mult, op1=Alu.add)
                    # out-projection: x_d = h @ w_out, + residual
                    if d == 0:
                        y0 = [sb.tile([128, L], f16, tag=f"y0_{c}",
                                      name=f"y0_{c}") for c in range(NC_D)]
                    else:
                        yt = [sb.tile([128, L], f16, tag=f"yt{c}",
                                      name=f"yt{c}") for c in range(NC_D)]
                    for m in range(NC_D):
                        q = ps.tile([128, L], f32, tag="bk", name=f"xq{m}")
                        for j in range(NC_H):
                            wj = go_t[j][:, m * 128:(m + 1) * 128]
                            for h in HS:
                                nc.tensor.matmul(
                                    q[:, h], wj, hh[j][:, h],
                                    start=(j == 0), stop=(j == NC_H - 1))
                        if d == 0:
                            nc.vector.tensor_tensor(
                                y0[m][:], q[:], xt[m][:], op=Alu.add)
                        else:
                            nc.vector.tensor_tensor(
                                yt[m][:], q[:], y0[m][:], op=Alu.add)

                # ---- LN2 + MLP ----
                tm2, rstd2 = layer_norm_stats(yt, "b")
                yn = [sb.tile([128, L], f16, tag=f"yn{c}", name=f"yn{c}")
                      for c in range(NC_D)]
                for c in range(NC_D):
                    if ln_trivial:
                        nc.vector.tensor_tensor(yn[c][:], tm2[c][:], rstd2[:],
                                                op=Alu.mult)
                    else:
                        nc.vector.scalar_tensor_tensor(
                            yn[c][:], tm2[c][:], gm2_t[:, c:c + 1], rstd2[:],
                            op0=Alu.mult, op1=Alu.mult)
                        nc.vector.tensor_scalar(
                            yn[c][:], yn[c][:], bt2_t[:, c:c + 1], None,
                            op0=Alu.add)
                yh = [sb.tile([128, L], f16, tag=f"yh{j}", name=f"yh{j}")
                      for j in range(NC_M)]
                for j in range(NC_M):
                    pph = ps.tile([128, L], f32, tag="bk", name=f"pp{j}")
                    for k in range(NC_D):
                        wj = p1w_t[k][:, j * 128:(j + 1) * 128]
                        for h in HS:
                            nc.tensor.matmul(
                                pph[:, h], wj, yn[k][:, h],
                                start=(k == 0), stop=(k == NC_D - 1))
                    nc.scalar.activation(yh[j][:], pph[:], Act.Gelu,
                                         bias=pb1_t[:, j:j + 1])
                yo = [sb.tile([128, L], f16, tag=f"y0_{c}", name=f"yo{c}")
                      for c in range(NC_D)]
                for m in range(NC_D):
                    q = ps.tile([128, L], f32, tag="bk", name=f"oq{m}")
                    for j in range(NC_M):
                        wj = p2w_t[j][:, m * 128:(m + 1) * 128]
                        for h in HS:
                            nc.tensor.matmul(
                                q[:, h], wj, yh[j][:, h],
                                start=(j == 0), stop=(j == NC_M - 1))
                    nc.vector.scalar_tensor_tensor(
                        yo[m][:], q[:], pb2_t[:, m:m + 1], yt[m][:],
                        op0=Alu.add, op1=Alu.add)
                for c in range(NC_D):
                    nc.sync.dma_start(y_d.ap()[s, c * 128:(c + 1) * 128, :],
                                      yo[c][:])

    nc.compile()
    return nc


_NC_CACHE = {}


def _get_nc(ns=NS, num_devices=N_CORES, ln_trivial=True):
    key = (ns, num_devices, ln_trivial)
    if key not in _NC_CACHE:
        _NC_CACHE[key] = build_nc(ns, num_devices, ln_trivial=ln_trivial)
    return _NC_CACHE[key]


def make_weight_maps(gamma1, beta1, dwc_w, dwc_b, gru1_w, gru1_out,
                     gru2_w, gru2_out, gamma2, beta2, p1_w, p1_b, p2_w, p2_b):
    f = np.float32
    dwc = np.asarray(dwc_w, f).reshape(DIM, 9)
    cdiag = np.zeros((9, NC_D, 128, 128), np.float16)
    for t in range(9):
        for c in range(NC_D):
            np.einsum('ii->i', cdiag[t, c])[:] = dwc[c * 128:(c + 1) * 128, t]
    col = lambda v: np.ascontiguousarray(
        np.asarray(v, f).reshape(NC_D if len(np.asarray(v).reshape(-1)) == DIM
                                 else NC_M, 128).T)
    return dict(
        g1w=np.asarray(gru1_w, f).astype(np.float16),
        g2w=np.asarray(gru2_w, f).astype(np.float16),
        g1o=np.asarray(gru1_out, f).astype(np.float16),
        g2o=np.asarray(gru2_out, f).astype(np.float16),
        p1w=np.asarray(p1_w, f).astype(np.float16),
        p2w=np.asarray(p2_w, f).astype(np.float16),
        cdiag=cdiag,
        pb1=col(p1_b), pb2=col(p2_b), dwcb=col(dwc_b),
        gm1=col(gamma1), bt1=col(beta1), gm2=col(gamma2), bt2=col(beta2),
    )


def kernel(x, gamma1, beta1, dwc_w, dwc_b, gru1_w, gru1_out, gru2_w, gru2_out,
           gamma2, beta2, p1_w, p1_b, p2_w, p2_b, h, w):
    x = np.asarray(x, np.float32)
    n = x.shape[0]
    ln_trivial = (np.allclose(np.asarray(gamma1), 1.0)
                  and np.allclose(np.asarray(beta1), 0.0)
                  and np.allclose(np.asarray(gamma2), 1.0)
                  and np.allclose(np.asarray(beta2), 0.0))
    nc = _get_nc(ln_trivial=ln_trivial)
    xT = np.ascontiguousarray(x.transpose(0, 2, 1)).astype(np.float16)
    wmap = make_weight_maps(gamma1, beta1, dwc_w, dwc_b, gru1_w, gru1_out,
                            gru2_w, gru2_out, gamma2, beta2, p1_w, p1_b,
                            p2_w, p2_b)
    in_maps = []
    for i in range(N_CORES):
        m = dict(wmap)
        m["xT"] = xT[i * NS:(i + 1) * NS]
        in_maps.append(m)
    res = run_bass_kernel_spmd(nc, in_maps, list(range(N_CORES)))
    yT = np.concatenate([res.results[i]["y"] for i in range(N_CORES)], axis=0)
    return np.ascontiguousarray(yT.transpose(0, 2, 1)).astype(np.float32)


# revision 11
# speedup vs baseline: 2.3510x; 1.1098x over previous
"""Trainium2 Bass kernel for nn_Block2DGRU: LN -> dw3x3 conv -> bidirectional
minGRU -> MLP, data-parallel over batch (32 samples -> 8 cores x 4).

Layout: per-sample transposed [d, L] (channels on partitions, sequence on
free dim). minGRU scan in linear space via DVE tensor_tensor_scan
(h_t = a_t*h_{t-1} + b_t); direction 2 via negative-stride APs.

v2: 2-bank PSUM tiles (FD=1024 ACT/DVE ops), weight-stationary-reuse matmul
order, separate LN1/LN2 temp tags + double-buffered GRU temps for
cross-sample overlap, f16 input/residual tiles.
"""
import numpy as np
import concourse.bacc as bacc
import concourse.tile as tile
import concourse.mybir as mybir
from concourse.bass_utils import run_bass_kernel_spmd

N_CORES = 8
NS = 4          # samples per core
DIM = 384
DI = 768        # minGRU inner dim
MLPD = 1536
L = 1024        # 32*32 flattened grid
GH = GW = 32
EPS = 1e-5
NC_D = DIM // 128    # 3 channel chunks
NC_H = DI // 128     # 6 hidden chunks
NC_M = MLPD // 128   # 12 mlp chunks

f32 = mybir.dt.float32
f32r = mybir.dt.float32r
f16 = mybir.dt.float16
Alu = mybir.AluOpType
Act = mybir.ActivationFunctionType

PADW = 34
PADN = PADW * PADW  # 1156
H0 = slice(0, 512)
H1 = slice(512, 1024)
HS = (H0, H1)


def build_nc(ns=NS, num_devices=N_CORES, ablate=(), ln_trivial=True,
             opts=()):
    nc = bacc.Bacc("TRN2", target_bir_lowering=False, debug=False,
                   num_devices=num_devices)

    # ---- DRAM I/O ----
    xT_d = nc.dram_tensor("xT", [ns, DIM, L], f16, kind="ExternalInput")
    g1w_d = nc.dram_tensor("g1w", [DIM, 2 * DI], f16, kind="ExternalInput")
    g2w_d = nc.dram_tensor("g2w", [DIM, 2 * DI], f16, kind="ExternalInput")
    g1o_d = nc.dram_tensor("g1o", [DI, DIM], f16, kind="ExternalInput")
    g2o_d = nc.dram_tensor("g2o", [DI, DIM], f16, kind="ExternalInput")
    p1w_d = nc.dram_tensor("p1w", [DIM, MLPD], f16, kind="ExternalInput")
    p2w_d = nc.dram_tensor("p2w", [MLPD, DIM], f16, kind="ExternalInput")
    cdiag_d = nc.dram_tensor("cdiag", [9, NC_D, 128, 128], f16,
                             kind="ExternalInput")
    pb1_d = nc.dram_tensor("pb1", [128, NC_M], f32, kind="ExternalInput")
    pb2_d = nc.dram_tensor("pb2", [128, NC_D], f32, kind="ExternalInput")
    dwcb_d = nc.dram_tensor("dwcb", [128, NC_D], f32, kind="ExternalInput")
    gm1_d = nc.dram_tensor("gm1", [128, NC_D], f32, kind="ExternalInput")
    bt1_d = nc.dram_tensor("bt1", [128, NC_D], f32, kind="ExternalInput")
    gm2_d = nc.dram_tensor("gm2", [128, NC_D], f32, kind="ExternalInput")
    bt2_d = nc.dram_tensor("bt2", [128, NC_D], f32, kind="ExternalInput")
    y_d = nc.dram_tensor("y", [ns, DIM, L], f16, kind="ExternalOutput")

    with tile.TileContext(nc) as tc:
        with nc.allow_low_precision(reason="f16 intermediates within 2e-2 tol"), \
             tc.tile_pool(name="wp", bufs=1) as wp, \
             tc.tile_pool(name="sb", bufs=1) as sb, \
             tc.tile_pool(name="ps", bufs=4, space="PSUM") as ps:

            # ---- persistent weights in SBUF ----
            onesf = wp.tile([128, 128], f16, tag="onesf", name="onesf")
            nc.vector.memset(onesf[:], 1.0)
            epsc = wp.tile([128, 1], f32, tag="epsc", name="epsc")
            nc.vector.memset(epsc[:], EPS)
            g1w_t = [wp.tile([128, 2 * DI], f16, tag=f"g1w{k}", name=f"g1w{k}") for k in range(NC_D)]
            g2w_t = [wp.tile([128, 2 * DI], f16, tag=f"g2w{k}", name=f"g2w{k}") for k in range(NC_D)]
            g1o_t = [wp.tile([128, DIM], f16, tag=f"g1o{j}", name=f"g1o{j}") for j in range(NC_H)]
            g2o_t = [wp.tile([128, DIM], f16, tag=f"g2o{j}", name=f"g2o{j}") for j in range(NC_H)]
            p1w_t = [wp.tile([128, MLPD], f16, tag=f"p1w{k}", name=f"p1w{k}") for k in range(NC_D)]
            p2w_t = [wp.tile([128, DIM], f16, tag=f"p2w{j}", name=f"p2w{j}") for j in range(NC_M)]
            cd_t = [[wp.tile([128, 128], f16, tag=f"cd{t}_{c}", name=f"cd{t}_{c}")
                     for c in range(NC_D)] for t in range(9)]
            pb1_t = wp.tile([128, NC_M], f32, tag="pb1", name="pb1")
            pb2_t = wp.tile([128, NC_D], f32, tag="pb2", name="pb2")
            dwcb_t = wp.tile([128, NC_D], f32, tag="dwcb", name="dwcb")
            gm1_t = wp.tile([128, NC_D], f32, tag="gm1", name="gm1")
            bt1_t = wp.tile([128, NC_D], f32, tag="bt1", name="bt1")
            gm2_t = wp.tile([128, NC_D], f32, tag="gm2", name="gm2")
            bt2_t = wp.tile([128, NC_D], f32, tag="bt2", name="bt2")

            for k in range(NC_D):
                nc.sync.dma_start(g1w_t[k][:], g1w_d.ap()[k * 128:(k + 1) * 128, :])
                nc.sync.dma_start(g2w_t[k][:], g2w_d.ap()[k * 128:(k + 1) * 128, :])
                nc.sync.dma_start(p1w_t[k][:], p1w_d.ap()[k * 128:(k + 1) * 128, :])
            for j in range(NC_H):
                nc.sync.dma_start(g1o_t[j][:], g1o_d.ap()[j * 128:(j + 1) * 128, :])
                nc.sync.dma_start(g2o_t[j][:], g2o_d.ap()[j * 128:(j + 1) * 128, :])
            for j in range(NC_M):
                nc.sync.dma_start(p2w_t[j][:], p2w_d.ap()[j * 128:(j + 1) * 128, :])
            for t in range(9):
                for c in range(NC_D):
                    nc.sync.dma_start(cd_t[t][c][:], cdiag_d.ap()[t, c])
            for name, tl, dr in [("pb1", pb1_t, pb1_d), ("pb2", pb2_t, pb2_d),
                                 ("dwcb", dwcb_t, dwcb_d), ("gm1", gm1_t, gm1_d),
                                 ("bt1", bt1_t, bt1_d), ("gm2", gm2_t, gm2_d),
                                 ("bt2", bt2_t, bt2_d)]:
                nc.sync.dma_start(tl[:], dr.ap()[:])

            def layer_norm_stats(src_tiles, tag):
                """src_tiles: 3 x [128,1024] f16. Returns (tm, rstd): f16
                (x-mu) chunk tiles and f16 [128,1024] rstd tile."""
                if "ln" in ablate:
                    tm = [sb.tile([128, L], f16, tag=f"tm{c}{tag}",
                                  name=f"tma{c}_{tag}") for c in range(NC_D)]
                    for c in range(NC_D):
                        nc.vector.memset(tm[c][:], 0.5)
                    rstd = sb.tile([128, L], f16, tag=f"rstd{tag}",
                                   name=f"rstd_{tag}")
                    nc.vector.memset(rstd[:], 1.0)
                    return tm, rstd
                Sh = ps.tile([128, L], f32, tag="bk", name=f"lnS_{tag}")
                SSh = ps.tile([128, L], f32, tag="bk", name=f"lnSS_{tag}")
                for c in range(NC_D):
                    for h in HS:
                        nc.tensor.matmul(Sh[:, h], onesf[:], src_tiles[c][:, h],
                                         start=(c == 0), stop=(c == NC_D - 1))
                sq = sb.tile([128, L], f16, tag=f"sq{tag}", name=f"sq_{tag}",
                             bufs=2)
                for c in range(NC_D):
                    nc.scalar.activation(sq[:], src_tiles[c][:], Act.Square)
                    for h in HS:
                        nc.tensor.matmul(SSh[:, h], onesf[:], sq[:, h],
                                         start=(c == 0), stop=(c == NC_D - 1))
                m2 = sb.tile([128, L], f32, tag=f"m2{tag}", name=f"m2_{tag}",
                             bufs=2)
                nc.scalar.activation(m2[:], Sh[:], Act.Square, scale=1.0 / DIM)
                tm = [sb.tile([128, L], f16, tag=f"tm{c}{tag}",
                              name=f"tm{c}_{tag}") for c in range(NC_D)]
                for c in range(NC_D):
                    nc.vector.scalar_tensor_tensor(
                        tm[c][:], Sh[:], -1.0 / DIM, src_tiles[c][:],
                        op0=Alu.mult, op1=Alu.add)
                v = sb.tile([128, L], f32, tag=f"v{tag}", name=f"v_{tag}")
                nc.vector.scalar_tensor_tensor(
                    v[:], SSh[:], 1.0 / DIM, m2[:],
                    op0=Alu.mult, op1=Alu.subtract)
                sd = sb.tile([128, L], f32, tag=f"m2{tag}", name=f"sd_{tag}",
                             bufs=2)
                nc.scalar.activation(sd[:], v[:], Act.Sqrt, bias=epsc[:])
                rstd = sb.tile([128, L], f16, tag=f"rstd{tag}",
                               name=f"rstd_{tag}")
                nc.vector.reciprocal(rstd[:], sd[:])
                return tm, rstd

            # Per-sample state for the software pipeline (MLP lags 1 sample).
            st = [dict() for _ in range(ns)]

            def emit_ln1(s):
                S = st[s]
                xt = [sb.tile([128, L], f16, tag=f"xt{c}", name=f"xt{c}_{s}",
                              bufs=2) for c in range(NC_D)]
                for c in range(NC_D):
                    nc.sync.dma_start(xt[c][:],
                                      xT_d.ap()[s, c * 128:(c + 1) * 128, :])
                S["xt"] = xt
                S["tm1"], S["rstd1"] = layer_norm_stats(xt, "a")
                xnp = [sb.tile([128, PADN], f16, tag=f"xnp{c}",
                               name=f"xnp{c}_{s}") for c in range(NC_D)]
                for c in range(NC_D):
                    nc.vector.memset(xnp[c][:], 0.0)
                    xnp3 = xnp[c][:].rearrange("p (a b) -> p a b", a=PADW)
                    t13 = S["tm1"][c][:].rearrange("p (a b) -> p a b", a=GH)
                    r13 = S["rstd1"][:].rearrange("p (a b) -> p a b", a=GH)
                    if ln_trivial:
                        nc.vector.tensor_tensor(
                            xnp3[:, 1:GH + 1, 1:GW + 1], t13[:, :, :],
                            r13[:, :, :], op=Alu.mult)
                    else:
                        nc.vector.scalar_tensor_tensor(
                            xnp3[:, 1:GH + 1, 1:GW + 1], t13[:, :, :],
                            gm1_t[:, c:c + 1], r13[:, :, :],
                            op0=Alu.mult, op1=Alu.mult)
                        nc.vector.tensor_scalar(
                            xnp3[:, 1:GH + 1, 1:GW + 1],
                            xnp3[:, 1:GH + 1, 1:GW + 1],
                            bt1_t[:, c:c + 1], None, op0=Alu.add)
                S["xnp"] = xnp

            def emit_conv(s):
                S = st[s]
                xs = [sb.tile([128, L], f16, tag=f"xs{c}", name=f"xs{c}_{s}")
                      for c in range(NC_D)]
                if "conv" in ablate:
                    for c in range(NC_D):
                        nc.vector.memset(xs[c][:], 0.01)
                for c in range(NC_D if "conv" not in ablate else 0):
                    xp3 = S["xnp"][c][:].rearrange("p (a b) -> p a b", a=PADW)
                    cv = ps.tile([128, L], f32, tag="bk", name=f"cv{c}_{s}")
                    for t in range(9):
                        ky, kx = divmod(t, 3)
                        for h in range(2):
                            rhs = xp3[:, ky + 16 * h: ky + 16 * h + 16,
                                      kx: kx + GW]
                            nc.tensor.matmul(cv[:, HS[h]], cd_t[t][c][:], rhs,
                                             start=(t == 0), stop=(t == 8))
                    nc.scalar.activation(xs[c][:], cv[:], Act.Identity,
                                         bias=dwcb_t[:, c:c + 1])
                S["xs"] = xs

            def emit_gru_dir(s, d):
                S = st[s]
                xs = S["xs"]
                gw_t = g1w_t if d == 0 else g2w_t
                go_t = g1o_t if d == 0 else g2o_t
                tagp = "hh" if d == 0 else "yh"
                hh = [sb.tile([128, L], f16, tag=f"{tagp}{j}",
                              name=f"hh{j}_{s}{d}") for j in range(NC_H)]
                for j in range(NC_H):
                    Hp = ps.tile([128, L], f32, tag="bk", name=f"Hp{j}_{s}{d}")
                    Gp = ps.tile([128, L], f32, tag="bk", name=f"Gp{j}_{s}{d}")
                    for k in range(NC_D):
                        wj = gw_t[k][:, j * 128:(j + 1) * 128]
                        for h in HS:
                            nc.tensor.matmul(
                                Hp[:, h], wj, xs[k][:, h],
                                start=(k == 0), stop=(k == NC_D - 1))
                    for k in range(NC_D):
                        wj = gw_t[k][:, (j + NC_H) * 128:(j + NC_H + 1) * 128]
                        for h in HS:
                            nc.tensor.matmul(
                                Gp[:, h], wj, xs[k][:, h],
                                start=(k == 0), stop=(k == NC_D - 1))
                    if "ew" in ablate:
                        nc.vector.memset(hh[j][:], 0.01)
                        continue
                    z = sb.tile([128, L], f16, tag="z", name=f"z{s}{d}{j}",
                                bufs=2)
                    sh = sb.tile([128, L], f16, tag="sh", name=f"sh{s}{d}{j}",
                                 bufs=2)
                    nc.scalar.activation(z[:], Gp[:], Act.Sigmoid)
                    nc.scalar.activation(sh[:], Hp[:], Act.Sigmoid)
                    # g(H) = relu(H) + min(sigmoid(H), 0.5) == max(H + 0.5,
                    # sigmoid(H)): one STT replaces relu + min-add.
                    g = sb.tile([128, L], f16, tag="g", name=f"g{s}{d}{j}",
                                bufs=2)
                    nc.vector.scalar_tensor_tensor(
                        g[:], Hp[:], 0.5, sh[:], op0=Alu.add, op1=Alu.max)
                    b = sb.tile([128, L], f16, tag="b", name=f"b{s}{d}{j}",
                                bufs=2)
                    a = sb.tile([128, L], f16, tag="a", name=f"a{s}{d}{j}",
                                bufs=2)
                    nc.vector.tensor_tensor(b[:], z[:], g[:], op=Alu.mult)
                    nc.vector.tensor_scalar(a[:], z[:], -1.0, 1.0,
                                            op0=Alu.mult, op1=Alu.add)
                    if "scan" in ablate:
                        nc.vector.tensor_copy(hh[j][:], b[:])
                    elif d == 0:
                        nc.vector.tensor_tensor_scan(
                            hh[j][:], a[:], b[:], 0.0,
                            op0=Alu.mult, op1=Alu.add)
                    else:
                        nc.vector.tensor_tensor_scan(
                            hh[j][:, ::-1], a[:, ::-1], b[:, ::-1], 0.0,
                            op0=Alu.mult, op1=Alu.add)
                S[f"hh{d}"] = hh

            def emit_outproj(s, d):
                S = st[s]
                go_t = g1o_t if d == 0 else g2o_t
                hh = S[f"hh{d}"]
                if d == 0:
                    dst = [sb.tile([128, L], f16, tag=f"y0_{c}",
                                   name=f"y0_{c}_{s}") for c in range(NC_D)]
                    res = S["xt"]
                    S["y0"] = dst
                else:
                    dst = [sb.tile([128, L], f16, tag=f"yt{c}",
                                   name=f"yt{c}_{s}") for c in range(NC_D)]
                    res = S["y0"]
                    S["yt"] = dst
                for m in range(NC_D):
                    q = ps.tile([128, L], f32, tag="bk", name=f"xq{m}_{s}{d}")
                    for j in range(NC_H):
                        wj = go_t[j][:, m * 128:(m + 1) * 128]
                        for h in HS:
                            nc.tensor.matmul(
                                q[:, h], wj, hh[j][:, h],
                                start=(j == 0), stop=(j == NC_H - 1))
                    nc.vector.tensor_tensor(dst[m][:], q[:], res[m][:],
                                            op=Alu.add)

            def emit_ln2(s):
                S = st[s]
                S["tm2"], S["rstd2"] = layer_norm_stats(S["yt"], "b")
                yn = [sb.tile([128, L], f16, tag=f"yn{c}", name=f"yn{c}_{s}")
                      for c in range(NC_D)]
                for c in range(NC_D):
                    if ln_trivial:
                        nc.vector.tensor_tensor(yn[c][:], S["tm2"][c][:],
                                                S["rstd2"][:], op=Alu.mult)
                    else:
                        nc.vector.scalar_tensor_tensor(
                            yn[c][:], S["tm2"][c][:], gm2_t[:, c:c + 1],
                            S["rstd2"][:], op0=Alu.mult, op1=Alu.mult)
                        nc.vector.tensor_scalar(
                            yn[c][:], yn[c][:], bt2_t[:, c:c + 1], None,
                            op0=Alu.add)
                S["yn"] = yn

            def emit_mlp(s):
                S = st[s]
                yn = S["yn"]
                yh = [sb.tile([128, L], f16, tag=f"yh{j}", name=f"yh{j}_{s}")
                      for j in range(NC_M)]
                for j in range(NC_M):
                    pph = ps.tile([128, L], f32, tag="bk", name=f"pp{j}_{s}")
                    for k in range(NC_D):
                        wj = p1w_t[k][:, j * 128:(j + 1) * 128]
                        for h in HS:
                            nc.tensor.matmul(
                                pph[:, h], wj, yn[k][:, h],
                                start=(k == 0), stop=(k == NC_D - 1))
                    nc.scalar.activation(yh[j][:], pph[:], Act.Gelu,
                                         bias=pb1_t[:, j:j + 1])
                yo = [sb.tile([128, L], f16, tag=f"y0_{c}", name=f"yo{c}_{s}")
                      for c in range(NC_D)]
                for m in range(NC_D):
                    q = ps.tile([128, L], f32, tag="bk", name=f"oq{m}_{s}")
                    for j in range(NC_M):
                        wj = p2w_t[j][:, m * 128:(m + 1) * 128]
                        for h in HS:
                            nc.tensor.matmul(
                                q[:, h], wj, yh[j][:, h],
                                start=(j == 0), stop=(j == NC_M - 1))
                    nc.vector.scalar_tensor_tensor(
                        yo[m][:], q[:], pb2_t[:, m:m + 1], S["yt"][m][:],
                        op0=Alu.add, op1=Alu.add)
                for c in range(NC_D):
                    nc.sync.dma_start(y_d.ap()[s, c * 128:(c + 1) * 128, :],
                                      yo[c][:])

            # Software pipeline: LN1 runs one sample ahead (its long ACT/DVE
            # chain resolves under MLP/GRU work), conv(s) uses last
            # iteration's xnp and covers the LN2(s-1)->yn chain before
            # MLP(s-1), whose matmuls in turn cover LN1(s+1)'s chain.
            emit_ln1(0)
            for s in range(ns):
                if s + 1 < ns:
                    emit_ln1(s + 1)
                emit_conv(s)
                if s >= 1:
                    emit_mlp(s - 1)
                emit_gru_dir(s, 0)
                emit_gru_dir(s, 1)   # hg(d1) fills PE while d0 scans run
                emit_outproj(s, 0)
                emit_outproj(s, 1)
                emit_ln2(s)
            emit_mlp(ns - 1)

    nc.compile()
    return nc


_NC_CACHE = {}


def _get_nc(ns=NS, num_devices=N_CORES, ln_trivial=True):
    key = (ns, num_devices, ln_trivial)
    if key not in _NC_CACHE:
        _NC_CACHE[key] = build_nc(ns, num_devices, ln_trivial=ln_trivial)
    return _NC_CACHE[key]


def make_weight_maps(gamma1, beta1, dwc_w, dwc_b, gru1_w, gru1_out,
                     gru2_w, gru2_out, gamma2, beta2, p1_w, p1_b, p2_w, p2_b):
    f = np.float32
    dwc = np.asarray(dwc_w, f).reshape(DIM, 9)
    cdiag = np.zeros((9, NC_D, 128, 128), np.float16)
    for t in range(9):
        for c in range(NC_D):
            np.einsum('ii->i', cdiag[t, c])[:] = dwc[c * 128:(c + 1) * 128, t]
    col = lambda v: np.ascontiguousarray(
        np.asarray(v, f).reshape(NC_D if len(np.asarray(v).reshape(-1)) == DIM
                                 else NC_M, 128).T)
    return dict(
        g1w=np.asarray(gru1_w, f).astype(np.float16),
        g2w=np.asarray(gru2_w, f).astype(np.float16),
        g1o=np.asarray(gru1_out, f).astype(np.float16),
        g2o=np.asarray(gru2_out, f).astype(np.float16),
        p1w=np.asarray(p1_w, f).astype(np.float16),
        p2w=np.asarray(p2_w, f).astype(np.float16),
        cdiag=cdiag,
        pb1=col(p1_b), pb2=col(p2_b), dwcb=col(dwc_b),
        gm1=col(gamma1), bt1=col(beta1), gm2=col(gamma2), bt2=col(beta2),
    )


def kernel(x, gamma1, beta1, dwc_w, dwc_b, gru1_w, gru1_out, gru2_w, gru2_out,
           gamma2, beta2, p1_w, p1_b, p2_w, p2_b, h, w):
    x = np.asarray(x, np.float32)
    n = x.shape[0]
    ln_trivial = (np.allclose(np.asarray(gamma1), 1.0)
                  and np.allclose(np.asarray(beta1), 0.0)
                  and np.allclose(np.asarray(gamma2), 1.0)
                  and np.allclose(np.asarray(beta2), 0.0))
    nc = _get_nc(ln_trivial=ln_trivial)
    xT = np.ascontiguousarray(x.transpose(0, 2, 1)).astype(np.float16)
    wmap = make_weight_maps(gamma1, beta1, dwc_w, dwc_b, gru1_w, gru1_out,
                            gru2_w, gru2_out, gamma2, beta2, p1_w, p1_b,
                            p2_w, p2_b)
    in_maps = []
    for i in range(N_CORES):
        m = dict(wmap)
        m["xT"] = xT[i * NS:(i + 1) * NS]
        in_maps.append(m)
    res = run_bass_kernel_spmd(nc, in_maps, list(range(N_CORES)))
    yT = np.concatenate([res.results[i]["y"] for i in range(N_CORES)], axis=0)
    return np.ascontiguousarray(yT.transpose(0, 2, 1)).astype(np.float32)


# revision 14
# speedup vs baseline: 3.3165x; 1.4107x over previous
"""Trainium2 Bass kernel for nn_Block2DGRU: LN -> dw3x3 conv -> bidirectional
minGRU -> MLP, data-parallel over batch (32 samples -> 8 cores x 4).

Layout: per-sample transposed [d, L] (channels on partitions, sequence on
free dim). minGRU scan in linear space via DVE tensor_tensor_scan
(h_t = a_t*h_{t-1} + b_t); direction 2 via negative-stride APs.

v2: 2-bank PSUM tiles (FD=1024 ACT/DVE ops), weight-stationary-reuse matmul
order, separate LN1/LN2 temp tags + double-buffered GRU temps for
cross-sample overlap, f16 input/residual tiles.
"""
import numpy as np
import concourse.bacc as bacc
import concourse.tile as tile
import concourse.mybir as mybir
from concourse.bass_utils import run_bass_kernel_spmd

N_CORES = 8
NS = 4          # samples per core
DIM = 384
DI = 768        # minGRU inner dim
MLPD = 1536
L = 1024        # 32*32 flattened grid
GH = GW = 32
EPS = 1e-5
NC_D = DIM // 128    # 3 channel chunks
NC_H = DI // 128     # 6 hidden chunks
NC_M = MLPD // 128   # 12 mlp chunks

f32 = mybir.dt.float32
f32r = mybir.dt.float32r
f16 = mybir.dt.float16
Alu = mybir.AluOpType
Act = mybir.ActivationFunctionType

PADW = 34
PADN = PADW * PADW  # 1156
H0 = slice(0, 512)
H1 = slice(512, 1024)
HS = (H0, H1)


DEFAULT_OPTS = ("gp_a", "gp_xnp")


def build_nc(ns=NS, num_devices=N_CORES, ablate=(), ln_trivial=True,
             opts=DEFAULT_OPTS):
    nc = bacc.Bacc("TRN2", target_bir_lowering=False, debug=False,
                   num_devices=num_devices)

    # ---- DRAM I/O ----
    xT_d = nc.dram_tensor("xT", [ns, DIM, L], f16, kind="ExternalInput")
    g1w_d = nc.dram_tensor("g1w", [DIM, 2 * DI], f16, kind="ExternalInput")
    g2w_d = nc.dram_tensor("g2w", [DIM, 2 * DI], f16, kind="ExternalInput")
    g1o_d = nc.dram_tensor("g1o", [DI, DIM], f16, kind="ExternalInput")
    g2o_d = nc.dram_tensor("g2o", [DI, DIM], f16, kind="ExternalInput")
    p1w_d = nc.dram_tensor("p1w", [DIM, MLPD], f16, kind="ExternalInput")
    p2w_d = nc.dram_tensor("p2w", [MLPD, DIM], f16, kind="ExternalInput")
    cdiag_d = nc.dram_tensor("cdiag", [9, NC_D, 128, 128], f16,
                             kind="ExternalInput")
    pb1_d = nc.dram_tensor("pb1", [128, NC_M], f32, kind="ExternalInput")
    pb2_d = nc.dram_tensor("pb2", [128, NC_D], f32, kind="ExternalInput")
    dwcb_d = nc.dram_tensor("dwcb", [128, NC_D], f32, kind="ExternalInput")
    gm1_d = nc.dram_tensor("gm1", [128, NC_D], f32, kind="ExternalInput")
    bt1_d = nc.dram_tensor("bt1", [128, NC_D], f32, kind="ExternalInput")
    gm2_d = nc.dram_tensor("gm2", [128, NC_D], f32, kind="ExternalInput")
    bt2_d = nc.dram_tensor("bt2", [128, NC_D], f32, kind="ExternalInput")
    y_d = nc.dram_tensor("y", [ns, DIM, L], f16, kind="ExternalOutput")

    with tile.TileContext(nc) as tc:
        with nc.allow_low_precision(reason="f16 intermediates within 2e-2 tol"), \
             tc.tile_pool(name="wp", bufs=1) as wp, \
             tc.tile_pool(name="sb", bufs=1) as sb, \
             tc.tile_pool(name="ps", bufs=4, space="PSUM") as ps:

            # ---- persistent weights in SBUF ----
            onesf = wp.tile([128, 128], f16, tag="onesf", name="onesf")
            nc.vector.memset(onesf[:], 1.0)
            epsc = wp.tile([128, 1], f32, tag="epsc", name="epsc")
            nc.vector.memset(epsc[:], EPS)
            g1w_t = [wp.tile([128, 2 * DI], f16, tag=f"g1w{k}", name=f"g1w{k}") for k in range(NC_D)]
            g2w_t = [wp.tile([128, 2 * DI], f16, tag=f"g2w{k}", name=f"g2w{k}") for k in range(NC_D)]
            g1o_t = [wp.tile([128, DIM], f16, tag=f"g1o{j}", name=f"g1o{j}") for j in range(NC_H)]
            g2o_t = [wp.tile([128, DIM], f16, tag=f"g2o{j}", name=f"g2o{j}") for j in range(NC_H)]
            p1w_t = [wp.tile([128, MLPD], f16, tag=f"p1w{k}", name=f"p1w{k}") for k in range(NC_D)]
            p2w_t = [wp.tile([128, DIM], f16, tag=f"p2w{j}", name=f"p2w{j}") for j in range(NC_M)]
            cd_t = [[wp.tile([128, 128], f16, tag=f"cd{t}_{c}", name=f"cd{t}_{c}")
                     for c in range(NC_D)] for t in range(9)]
            pb1_t = wp.tile([128, NC_M], f32, tag="pb1", name="pb1")
            pb2_t = wp.tile([128, NC_D], f32, tag="pb2", name="pb2")
            dwcb_t = wp.tile([128, NC_D], f32, tag="dwcb", name="dwcb")
            gm1_t = wp.tile([128, NC_D], f32, tag="gm1", name="gm1")
            bt1_t = wp.tile([128, NC_D], f32, tag="bt1", name="bt1")
            gm2_t = wp.tile([128, NC_D], f32, tag="gm2", name="gm2")
            bt2_t = wp.tile([128, NC_D], f32, tag="bt2", name="bt2")

            for k in range(NC_D):
                nc.sync.dma_start(g1w_t[k][:], g1w_d.ap()[k * 128:(k + 1) * 128, :])
                nc.sync.dma_start(g2w_t[k][:], g2w_d.ap()[k * 128:(k + 1) * 128, :])
                nc.sync.dma_start(p1w_t[k][:], p1w_d.ap()[k * 128:(k + 1) * 128, :])
            for j in range(NC_H):
                nc.sync.dma_start(g1o_t[j][:], g1o_d.ap()[j * 128:(j + 1) * 128, :])
                nc.sync.dma_start(g2o_t[j][:], g2o_d.ap()[j * 128:(j + 1) * 128, :])
            for j in range(NC_M):
                nc.sync.dma_start(p2w_t[j][:], p2w_d.ap()[j * 128:(j + 1) * 128, :])
            for t in range(9):
                for c in range(NC_D):
                    nc.sync.dma_start(cd_t[t][c][:], cdiag_d.ap()[t, c])
            for name, tl, dr in [("pb1", pb1_t, pb1_d), ("pb2", pb2_t, pb2_d),
                                 ("dwcb", dwcb_t, dwcb_d), ("gm1", gm1_t, gm1_d),
                                 ("bt1", bt1_t, bt1_d), ("gm2", gm2_t, gm2_d),
                                 ("bt2", bt2_t, bt2_d)]:
                nc.sync.dma_start(tl[:], dr.ap()[:])

            def layer_norm_stats(src_tiles, tag):
                """src_tiles: 3 x [128,1024] f16. Returns (tm, rstd): f16
                (x-mu) chunk tiles and f16 [128,1024] rstd tile."""
                if "ln" in ablate:
                    tm = [sb.tile([128, L], f16, tag=f"tm{c}{tag}",
                                  name=f"tma{c}_{tag}") for c in range(NC_D)]
                    for c in range(NC_D):
                        nc.vector.memset(tm[c][:], 0.5)
                    rstd = sb.tile([128, L], f16, tag=f"rstd{tag}",
                                   name=f"rstd_{tag}")
                    nc.vector.memset(rstd[:], 1.0)
                    return tm, rstd
                Sh = ps.tile([128, L], f32, tag="bk", name=f"lnS_{tag}")
                SSh = ps.tile([128, L], f32, tag="bk", name=f"lnSS_{tag}")
                for c in range(NC_D):
                    for h in HS:
                        nc.tensor.matmul(Sh[:, h], onesf[:], src_tiles[c][:, h],
                                         start=(c == 0), stop=(c == NC_D - 1))
                sq = sb.tile([128, L], f16, tag=f"sq{tag}", name=f"sq_{tag}",
                             bufs=2)
                for c in range(NC_D):
                    nc.scalar.activation(sq[:], src_tiles[c][:], Act.Square)
                    for h in HS:
                        nc.tensor.matmul(SSh[:, h], onesf[:], sq[:, h],
                                         start=(c == 0), stop=(c == NC_D - 1))
                m2 = sb.tile([128, L], f32, tag=f"m2{tag}", name=f"m2_{tag}",
                             bufs=2)
                nc.scalar.activation(m2[:], Sh[:], Act.Square, scale=1.0 / DIM)
                tm = [sb.tile([128, L], f16, tag=f"tm{c}{tag}",
                              name=f"tm{c}_{tag}") for c in range(NC_D)]
                for c in range(NC_D):
                    nc.vector.scalar_tensor_tensor(
                        tm[c][:], Sh[:], -1.0 / DIM, src_tiles[c][:],
                        op0=Alu.mult, op1=Alu.add)
                v = sb.tile([128, L], f32, tag=f"v{tag}", name=f"v_{tag}")
                nc.vector.scalar_tensor_tensor(
                    v[:], SSh[:], 1.0 / DIM, m2[:],
                    op0=Alu.mult, op1=Alu.subtract)
                sd = sb.tile([128, L], f32, tag=f"m2{tag}", name=f"sd_{tag}",
                             bufs=2)
                nc.scalar.activation(sd[:], v[:], Act.Sqrt, bias=epsc[:])
                rstd = sb.tile([128, L], f16, tag=f"rstd{tag}",
                               name=f"rstd_{tag}")
                nc.vector.reciprocal(rstd[:], sd[:])
                return tm, rstd

            # Per-sample state for the software pipeline (MLP lags 1 sample).
            st = [dict() for _ in range(ns)]

            def emit_ln1(s):
                S = st[s]
                xt = [sb.tile([128, L], f16, tag=f"xt{c}", name=f"xt{c}_{s}",
                              bufs=2) for c in range(NC_D)]
                for c in range(NC_D):
                    nc.sync.dma_start(xt[c][:],
                                      xT_d.ap()[s, c * 128:(c + 1) * 128, :])
                S["xt"] = xt
                S["tm1"], S["rstd1"] = layer_norm_stats(xt, "a")
                xnp = [sb.tile([128, PADN], f16, tag=f"xnp{c}",
                               name=f"xnp{c}_{s}") for c in range(NC_D)]
                for c in range(NC_D):
                    nc.vector.memset(xnp[c][:], 0.0)
                    xnp3 = xnp[c][:].rearrange("p (a b) -> p a b", a=PADW)
                    t13 = S["tm1"][c][:].rearrange("p (a b) -> p a b", a=GH)
                    r13 = S["rstd1"][:].rearrange("p (a b) -> p a b", a=GH)
                    if ln_trivial:
                        eng = nc.gpsimd if "gp_xnp" in opts else nc.vector
                        eng.tensor_tensor(
                            xnp3[:, 1:GH + 1, 1:GW + 1], t13[:, :, :],
                            r13[:, :, :], op=Alu.mult)
                    else:
                        nc.vector.scalar_tensor_tensor(
                            xnp3[:, 1:GH + 1, 1:GW + 1], t13[:, :, :],
                            gm1_t[:, c:c + 1], r13[:, :, :],
                            op0=Alu.mult, op1=Alu.mult)
                        nc.vector.tensor_scalar(
                            xnp3[:, 1:GH + 1, 1:GW + 1],
                            xnp3[:, 1:GH + 1, 1:GW + 1],
                            bt1_t[:, c:c + 1], None, op0=Alu.add)
                S["xnp"] = xnp

            def emit_conv(s):
                S = st[s]
                xs = [sb.tile([128, L], f16, tag=f"xs{c}", name=f"xs{c}_{s}")
                      for c in range(NC_D)]
                if "conv" in ablate:
                    for c in range(NC_D):
                        nc.vector.memset(xs[c][:], 0.01)
                for c in range(NC_D if "conv" not in ablate else 0):
                    xp3 = S["xnp"][c][:].rearrange("p (a b) -> p a b", a=PADW)
                    cv = ps.tile([128, L], f32, tag="bk", name=f"cv{c}_{s}")
                    for t in range(9):
                        ky, kx = divmod(t, 3)
                        for h in range(2):
                            rhs = xp3[:, ky + 16 * h: ky + 16 * h + 16,
                                      kx: kx + GW]
                            nc.tensor.matmul(cv[:, HS[h]], cd_t[t][c][:], rhs,
                                             start=(t == 0), stop=(t == 8))
                    nc.scalar.activation(xs[c][:], cv[:], Act.Identity,
                                         bias=dwcb_t[:, c:c + 1])
                S["xs"] = xs

            def emit_gru_dir(s, d):
                S = st[s]
                xs = S["xs"]
                gw_t = g1w_t if d == 0 else g2w_t
                go_t = g1o_t if d == 0 else g2o_t
                tagp = "hh" if d == 0 else "yh"
                hh = [sb.tile([128, L], f16, tag=f"{tagp}{j}",
                              name=f"hh{j}_{s}{d}") for j in range(NC_H)]
                for j in range(NC_H):
                    Hp = ps.tile([128, L], f32, tag="bk", name=f"Hp{j}_{s}{d}")
                    Gp = ps.tile([128, L], f32, tag="bk", name=f"Gp{j}_{s}{d}")
                    for k in range(NC_D):
                        wj = gw_t[k][:, j * 128:(j + 1) * 128]
                        for h in HS:
                            nc.tensor.matmul(
                                Hp[:, h], wj, xs[k][:, h],
                                start=(k == 0), stop=(k == NC_D - 1))
                    for k in range(NC_D):
                        wj = gw_t[k][:, (j + NC_H) * 128:(j + NC_H + 1) * 128]
                        for h in HS:
                            nc.tensor.matmul(
                                Gp[:, h], wj, xs[k][:, h],
                                start=(k == 0), stop=(k == NC_D - 1))
                    if "ew" in ablate:
                        nc.vector.memset(hh[j][:], 0.01)
                        continue
                    z = sb.tile([128, L], f16, tag="z", name=f"z{s}{d}{j}",
                                bufs=2)
                    sh = sb.tile([128, L], f16, tag="sh", name=f"sh{s}{d}{j}",
                                 bufs=2)
                    nc.scalar.activation(z[:], Gp[:], Act.Sigmoid)
                    nc.scalar.activation(sh[:], Hp[:], Act.Sigmoid)
                    # g(H) = relu(H) + min(sigmoid(H), 0.5) == max(H + 0.5,
                    # sigmoid(H)): one STT replaces relu + min-add.
                    g = sb.tile([128, L], f16, tag="g", name=f"g{s}{d}{j}",
                                bufs=2)
                    nc.vector.scalar_tensor_tensor(
                        g[:], Hp[:], 0.5, sh[:], op0=Alu.add, op1=Alu.max)
                    b = sb.tile([128, L], f16, tag="b", name=f"b{s}{d}{j}",
                                bufs=2)
                    a = sb.tile([128, L], f16, tag="a", name=f"a{s}{d}{j}",
                                bufs=2)
                    if "gp_ab" in opts:
                        nc.gpsimd.tensor_tensor(b[:], z[:], g[:], op=Alu.mult)
                        nc.gpsimd.tensor_scalar(a[:], z[:], -1.0, 1.0,
                                                op0=Alu.mult, op1=Alu.add)
                    elif "gp_a" in opts:
                        nc.vector.tensor_tensor(b[:], z[:], g[:], op=Alu.mult)
                        nc.gpsimd.tensor_scalar(a[:], z[:], -1.0, 1.0,
                                                op0=Alu.mult, op1=Alu.add)
                    else:
                        nc.vector.tensor_tensor(b[:], z[:], g[:], op=Alu.mult)
                        nc.vector.tensor_scalar(a[:], z[:], -1.0, 1.0,
                                                op0=Alu.mult, op1=Alu.add)
                    if "scan" in ablate:
                        nc.vector.tensor_copy(hh[j][:], b[:])
                    elif d == 0:
                        nc.vector.tensor_tensor_scan(
                            hh[j][:], a[:], b[:], 0.0,
                            op0=Alu.mult, op1=Alu.add)
                    else:
                        nc.vector.tensor_tensor_scan(
                            hh[j][:, ::-1], a[:, ::-1], b[:, ::-1], 0.0,
                            op0=Alu.mult, op1=Alu.add)
                S[f"hh{d}"] = hh

            def emit_outproj(s, d):
                S = st[s]
                go_t = g1o_t if d == 0 else g2o_t
                hh = S[f"hh{d}"]
                if d == 0:
                    dst = [sb.tile([128, L], f16, tag=f"y0_{c}",
                                   name=f"y0_{c}_{s}") for c in range(NC_D)]
                    res = S["xt"]
                    S["y0"] = dst
                else:
                    dst = [sb.tile([128, L], f16, tag=f"yt{c}",
                                   name=f"yt{c}_{s}") for c in range(NC_D)]
                    res = S["y0"]
                    S["yt"] = dst
                for m in range(NC_D):
                    q = ps.tile([128, L], f32, tag="bk", name=f"xq{m}_{s}{d}")
                    for j in range(NC_H):
                        wj = go_t[j][:, m * 128:(m + 1) * 128]
                        for h in HS:
                            nc.tensor.matmul(
                                q[:, h], wj, hh[j][:, h],
                                start=(j == 0), stop=(j == NC_H - 1))
                    nc.vector.tensor_tensor(dst[m][:], q[:], res[m][:],
                                            op=Alu.add)

            def emit_ln2(s):
                S = st[s]
                S["tm2"], S["rstd2"] = layer_norm_stats(S["yt"], "b")
                yn = [sb.tile([128, L], f16, tag=f"yn{c}", name=f"yn{c}_{s}")
                      for c in range(NC_D)]
                for c in range(NC_D):
                    if ln_trivial:
                        nc.vector.tensor_tensor(yn[c][:], S["tm2"][c][:],
                                                S["rstd2"][:], op=Alu.mult)
                    else:
                        nc.vector.scalar_tensor_tensor(
                            yn[c][:], S["tm2"][c][:], gm2_t[:, c:c + 1],
                            S["rstd2"][:], op0=Alu.mult, op1=Alu.mult)
                        nc.vector.tensor_scalar(
                            yn[c][:], yn[c][:], bt2_t[:, c:c + 1], None,
                            op0=Alu.add)
                S["yn"] = yn

            def emit_mlp(s):
                S = st[s]
                yn = S["yn"]
                yh = [sb.tile([128, L], f16, tag=f"yh{j}", name=f"yh{j}_{s}")
                      for j in range(NC_M)]
                for j in range(NC_M):
                    pph = ps.tile([128, L], f32, tag="bk", name=f"pp{j}_{s}")
                    for k in range(NC_D):
                        wj = p1w_t[k][:, j * 128:(j + 1) * 128]
                        for h in HS:
                            nc.tensor.matmul(
                                pph[:, h], wj, yn[k][:, h],
                                start=(k == 0), stop=(k == NC_D - 1))
                    nc.scalar.activation(yh[j][:], pph[:], Act.Gelu,
                                         bias=pb1_t[:, j:j + 1])
                yo = [sb.tile([128, L], f16, tag=f"y0_{c}", name=f"yo{c}_{s}")
                      for c in range(NC_D)]
                for m in range(NC_D):
                    q = ps.tile([128, L], f32, tag="bk", name=f"oq{m}_{s}")
                    for j in range(NC_M):
                        wj = p2w_t[j][:, m * 128:(m + 1) * 128]
                        for h in HS:
                            nc.tensor.matmul(
                                q[:, h], wj, yh[j][:, h],
                                start=(j == 0), stop=(j == NC_M - 1))
                    nc.vector.scalar_tensor_tensor(
                        yo[m][:], q[:], pb2_t[:, m:m + 1], S["yt"][m][:],
                        op0=Alu.add, op1=Alu.add)
                for c in range(NC_D):
                    nc.sync.dma_start(y_d.ap()[s, c * 128:(c + 1) * 128, :],
                                      yo[c][:])

            # Software pipeline: LN1 runs one sample ahead (its long ACT/DVE
            # chain resolves under MLP/GRU work), conv(s) uses last
            # iteration's xnp and covers the LN2(s-1)->yn chain before
            # MLP(s-1), whose matmuls in turn cover LN1(s+1)'s chain.
            emit_ln1(0)
            for s in range(ns):
                if s + 1 < ns:
                    emit_ln1(s + 1)
                emit_conv(s)
                if s >= 1:
                    emit_mlp(s - 1)
                emit_gru_dir(s, 0)
                emit_gru_dir(s, 1)   # hg(d1) fills PE while d0 scans run
                emit_outproj(s, 0)
                emit_outproj(s, 1)
                emit_ln2(s)
            emit_mlp(ns - 1)

    nc.compile()
    return nc


_NC_CACHE = {}


def _get_nc(ns=NS, num_devices=N_CORES, ln_trivial=True):
    key = (ns, num_devices, ln_trivial)
    if key not in _NC_CACHE:
        _NC_CACHE[key] = build_nc(ns, num_devices, ln_trivial=ln_trivial)
    return _NC_CACHE[key]


def make_weight_maps(gamma1, beta1, dwc_w, dwc_b, gru1_w, gru1_out,
                     gru2_w, gru2_out, gamma2, beta2, p1_w, p1_b, p2_w, p2_b):
    f = np.float32
    dwc = np.asarray(dwc_w, f).reshape(DIM, 9)
    cdiag = np.zeros((9, NC_D, 128, 128), np.float16)
    for t in range(9):
        for c in range(NC_D):
            np.einsum('ii->i', cdiag[t, c])[:] = dwc[c * 128:(c + 1) * 128, t]
    col = lambda v: np.ascontiguousarray(
        np.asarray(v, f).reshape(NC_D if len(np.asarray(v).reshape(-1)) == DIM
                                 else NC_M, 128).T)
    return dict(
        g1w=np.asarray(gru1_w, f).astype(np.float16),
        g2w=np.asarray(gru2_w, f).astype(np.float16),
        g1o=np.asarray(gru1_out, f).astype(np.float16),
        g2o=np.asarray(gru2_out, f).astype(np.float16),
        p1w=np.asarray(p1_w, f).astype(np.float16),
        p2w=np.asarray(p2_w, f).astype(np.float16),
        cdiag=cdiag,
        pb1=col(p1_b), pb2=col(p2_b), dwcb=col(dwc_b),
        gm1=col(gamma1), bt1=col(beta1), gm2=col(gamma2), bt2=col(beta2),
    )


def kernel(x, gamma1, beta1, dwc_w, dwc_b, gru1_w, gru1_out, gru2_w, gru2_out,
           gamma2, beta2, p1_w, p1_b, p2_w, p2_b, h, w):
    x = np.asarray(x, np.float32)
    n = x.shape[0]
    ln_trivial = (np.allclose(np.asarray(gamma1), 1.0)
                  and np.allclose(np.asarray(beta1), 0.0)
                  and np.allclose(np.asarray(gamma2), 1.0)
                  and np.allclose(np.asarray(beta2), 0.0))
    nc = _get_nc(ln_trivial=ln_trivial)
    xT = np.ascontiguousarray(x.transpose(0, 2, 1)).astype(np.float16)
    wmap = make_weight_maps(gamma1, beta1, dwc_w, dwc_b, gru1_w, gru1_out,
                            gru2_w, gru2_out, gamma2, beta2, p1_w, p1_b,
                            p2_w, p2_b)
    in_maps = []
    for i in range(N_CORES):
        m = dict(wmap)
        m["xT"] = xT[i * NS:(i + 1) * NS]
        in_maps.append(m)
    res = run_bass_kernel_spmd(nc, in_maps, list(range(N_CORES)))
    yT = np.concatenate([res.results[i]["y"] for i in range(N_CORES)], axis=0)
    return np.ascontiguousarray(yT.transpose(0, 2, 1)).astype(np.float32)


# revision 28
# speedup vs baseline: 3.8648x; 1.1653x over previous
"""Trainium2 Bass kernel for nn_Block2DGRU: LN -> dw3x3 conv -> bidirectional
minGRU -> MLP, data-parallel over batch (32 samples -> 8 cores x 4).

Layout: per-sample transposed [d, L] (channels on partitions, sequence on
free dim). minGRU scan in linear space via DVE tensor_tensor_scan
(h_t = a_t*h_{t-1} + b_t); direction 2 via negative-stride APs.

v2: 2-bank PSUM tiles (FD=1024 ACT/DVE ops), weight-stationary-reuse matmul
order, separate LN1/LN2 temp tags + double-buffered GRU temps for
cross-sample overlap, f16 input/residual tiles.
"""
import numpy as np
import concourse.bacc as bacc
import concourse.tile as tile
import concourse.mybir as mybir
from concourse.bass_utils import run_bass_kernel_spmd

N_CORES = 8
NS = 4          # samples per core
DIM = 384
DI = 768        # minGRU inner dim
MLPD = 1536
L = 1024        # 32*32 flattened grid
GH = GW = 32
EPS = 1e-5
NC_D = DIM // 128    # 3 channel chunks
NC_H = DI // 128     # 6 hidden chunks
NC_M = MLPD // 128   # 12 mlp chunks

f32 = mybir.dt.float32
f32r = mybir.dt.float32r
f16 = mybir.dt.float16
f8 = mybir.dt.float8e4
Alu = mybir.AluOpType
Act = mybir.ActivationFunctionType
PM = mybir.MatmulPerfMode
# fp8 weight scale: 1.0 — for 0.02-scale weights the e4m3 subnormal step
# (2^-9) equals the x64-scaled step /64, so scaling buys nothing and
# unscaled weights avoid a per-chunk rescale op on DVE.
WSC = 1.0

PADW = 34
PADN = PADW * PADW  # 1156
H0 = slice(0, 512)
H1 = slice(512, 1024)
HS = (H0, H1)


DEFAULT_OPTS = ("gp_a", "gp_xnp")


def build_nc(ns=NS, num_devices=N_CORES, ablate=(), ln_trivial=True,
             opts=DEFAULT_OPTS, pb2_zero=True):
    nc = bacc.Bacc("TRN2", target_bir_lowering=False, debug=False,
                   num_devices=num_devices)

    # ---- DRAM I/O ----
    xT_d = nc.dram_tensor("xT", [ns, DIM, L], f16, kind="ExternalInput")
    g1wp_d = nc.dram_tensor("g1wp", [128, 2, 2 * DI], f8, kind="ExternalInput")
    g2wp_d = nc.dram_tensor("g2wp", [128, 2, 2 * DI], f8, kind="ExternalInput")
    g1w2_d = nc.dram_tensor("g1w2", [128, 2 * DI], f8, kind="ExternalInput")
    g2w2_d = nc.dram_tensor("g2w2", [128, 2 * DI], f8, kind="ExternalInput")
    g1o_d = nc.dram_tensor("g1o", [DI, DIM], f16, kind="ExternalInput")
    g2o_d = nc.dram_tensor("g2o", [DI, DIM], f16, kind="ExternalInput")
    p1w_d = nc.dram_tensor("p1w", [DIM, MLPD], f16, kind="ExternalInput")
    p2w8_d = nc.dram_tensor("p2w8", [NC_M // 2, 128, 2, DIM], f8,
                            kind="ExternalInput")
    cdiag_d = nc.dram_tensor("cdiag", [9, NC_D, 128, 128], f16,
                             kind="ExternalInput")
    pb1_d = nc.dram_tensor("pb1", [128, NC_M], f32, kind="ExternalInput")
    pb2_d = nc.dram_tensor("pb2", [128, NC_D], f32, kind="ExternalInput")
    dwcb_d = nc.dram_tensor("dwcb", [128, NC_D], f32, kind="ExternalInput")
    gm1_d = nc.dram_tensor("gm1", [128, NC_D], f32, kind="ExternalInput")
    bt1_d = nc.dram_tensor("bt1", [128, NC_D], f32, kind="ExternalInput")
    gm2_d = nc.dram_tensor("gm2", [128, NC_D], f32, kind="ExternalInput")
    bt2_d = nc.dram_tensor("bt2", [128, NC_D], f32, kind="ExternalInput")
    y_d = nc.dram_tensor("y", [ns, DIM, L], f16, kind="ExternalOutput")

    with tile.TileContext(nc) as tc:
        with nc.allow_low_precision(reason="f16 intermediates within 2e-2 tol"), \
             tc.tile_pool(name="wp", bufs=1) as wp, \
             tc.tile_pool(name="sb", bufs=1) as sb, \
             tc.tile_pool(name="ps", bufs=4, space="PSUM") as ps:

            # ---- persistent weights in SBUF ----
            onesf = wp.tile([128, 128], f16, tag="onesf", name="onesf")
            nc.vector.memset(onesf[:], 1.0)
            epsc = wp.tile([128, 1], f32, tag="epsc", name="epsc")
            nc.vector.memset(epsc[:], EPS)
            g1wp_t = wp.tile([128, 2 * 2 * DI], f8, tag="g1wp", name="g1wp")
            g2wp_t = wp.tile([128, 2 * 2 * DI], f8, tag="g2wp", name="g2wp")
            g1w2_t = wp.tile([128, 2 * DI], f8, tag="g1w2", name="g1w2")
            g2w2_t = wp.tile([128, 2 * DI], f8, tag="g2w2", name="g2w2")
            g1o_t = [wp.tile([128, DIM], f16, tag=f"g1o{j}", name=f"g1o{j}") for j in range(NC_H)]
            g2o_t = [wp.tile([128, DIM], f16, tag=f"g2o{j}", name=f"g2o{j}") for j in range(NC_H)]
            p1w_t = [wp.tile([128, MLPD], f16, tag=f"p1w{k}", name=f"p1w{k}") for k in range(NC_D)]
            p2w8_t = [wp.tile([128, 2 * DIM], f8, tag=f"p2w8_{u}",
                              name=f"p2w8_{u}") for u in range(NC_M // 2)]
            cd_t = [[wp.tile([128, 128], f16, tag=f"cd{t}_{c}", name=f"cd{t}_{c}")
                     for c in range(NC_D)] for t in range(9)]
            pb1_t = wp.tile([128, NC_M], f32, tag="pb1", name="pb1")
            pb2_t = wp.tile([128, NC_D], f32, tag="pb2", name="pb2")
            dwcb_t = wp.tile([128, NC_D], f32, tag="dwcb", name="dwcb")
            gm1_t = wp.tile([128, NC_D], f32, tag="gm1", name="gm1")
            bt1_t = wp.tile([128, NC_D], f32, tag="bt1", name="bt1")
            gm2_t = wp.tile([128, NC_D], f32, tag="gm2", name="gm2")
            bt2_t = wp.tile([128, NC_D], f32, tag="bt2", name="bt2")

            nc.sync.dma_start(g1wp_t[:], g1wp_d.ap().rearrange("p i c -> p (i c)"))
            nc.sync.dma_start(g2wp_t[:], g2wp_d.ap().rearrange("p i c -> p (i c)"))
            nc.sync.dma_start(g1w2_t[:], g1w2_d.ap()[:])
            nc.sync.dma_start(g2w2_t[:], g2w2_d.ap()[:])
            for k in range(NC_D):
                nc.sync.dma_start(p1w_t[k][:], p1w_d.ap()[k * 128:(k + 1) * 128, :])
            for j in range(NC_H):
                nc.sync.dma_start(g1o_t[j][:], g1o_d.ap()[j * 128:(j + 1) * 128, :])
                nc.sync.dma_start(g2o_t[j][:], g2o_d.ap()[j * 128:(j + 1) * 128, :])
            for u in range(NC_M // 2):
                nc.sync.dma_start(p2w8_t[u][:],
                                  p2w8_d.ap()[u].rearrange("p i m -> p (i m)"))
            for t in range(9):
                for c in range(NC_D):
                    nc.sync.dma_start(cd_t[t][c][:], cdiag_d.ap()[t, c])
            for name, tl, dr in [("pb1", pb1_t, pb1_d), ("pb2", pb2_t, pb2_d),
                                 ("dwcb", dwcb_t, dwcb_d), ("gm1", gm1_t, gm1_d),
                                 ("bt1", bt1_t, bt1_d), ("gm2", gm2_t, gm2_d),
                                 ("bt2", bt2_t, bt2_d)]:
                nc.sync.dma_start(tl[:], dr.ap()[:])

            def layer_norm_stats(src_tiles, tag):
                """src_tiles: 3 x [128,1024] f16. Returns (tm, rstd): f16
                (x-mu) chunk tiles and f16 [128,1024] rstd tile."""
                if "ln" in ablate:
                    tm = [sb.tile([128, L], f16, tag=f"tm{c}{tag}",
                                  name=f"tma{c}_{tag}") for c in range(NC_D)]
                    for c in range(NC_D):
                        nc.vector.memset(tm[c][:], 0.5)
                    rstd = sb.tile([128, L], f16, tag=f"rstd{tag}",
                                   name=f"rstd_{tag}")
                    nc.vector.memset(rstd[:], 1.0)
                    return tm, rstd
                Sh = ps.tile([128, L], f32, tag="bk", name=f"lnS_{tag}")
                SSh = ps.tile([128, L], f32, tag="bk", name=f"lnSS_{tag}")
                for c in range(NC_D):
                    for h in HS:
                        nc.tensor.matmul(Sh[:, h], onesf[:], src_tiles[c][:, h],
                                         start=(c == 0), stop=(c == NC_D - 1))
                sq = sb.tile([128, L], f16, tag=f"sq{tag}", name=f"sq_{tag}",
                             bufs=2)
                for c in range(NC_D):
                    nc.scalar.activation(sq[:], src_tiles[c][:], Act.Square)
                    for h in HS:
                        nc.tensor.matmul(SSh[:, h], onesf[:], sq[:, h],
                                         start=(c == 0), stop=(c == NC_D - 1))
                m2 = sb.tile([128, L], f32, tag=f"m2{tag}", name=f"m2_{tag}",
                             bufs=2)
                nc.scalar.activation(m2[:], Sh[:], Act.Square, scale=1.0 / DIM)
                tm = [sb.tile([128, L], f16, tag=f"tm{c}{tag}",
                              name=f"tm{c}_{tag}") for c in range(NC_D)]
                for c in range(NC_D):
                    nc.vector.scalar_tensor_tensor(
                        tm[c][:], Sh[:], -1.0 / DIM, src_tiles[c][:],
                        op0=Alu.mult, op1=Alu.add)
                v = sb.tile([128, L], f32, tag=f"v{tag}", name=f"v_{tag}")
                nc.vector.scalar_tensor_tensor(
                    v[:], SSh[:], 1.0 / DIM, m2[:],
                    op0=Alu.mult, op1=Alu.subtract)
                sd = sb.tile([128, L], f32, tag=f"m2{tag}", name=f"sd_{tag}",
                             bufs=2)
                nc.scalar.activation(sd[:], v[:], Act.Sqrt, bias=epsc[:])
                rstd = sb.tile([128, L], f16, tag=f"rstd{tag}",
                               name=f"rstd_{tag}")
                nc.vector.reciprocal(rstd[:], sd[:])
                return tm, rstd

            # Per-sample state for the software pipeline (MLP lags 1 sample).
            st = [dict() for _ in range(ns)]

            def emit_ln1(s):
                S = st[s]
                xt = [sb.tile([128, L], f16, tag=f"xt{c}", name=f"xt{c}_{s}",
                              bufs=2) for c in range(NC_D)]
                for c in range(NC_D):
                    nc.sync.dma_start(xt[c][:],
                                      xT_d.ap()[s, c * 128:(c + 1) * 128, :])
                S["xt"] = xt
                S["tm1"], S["rstd1"] = layer_norm_stats(xt, "a")
                xnp = [sb.tile([128, PADN], f16, tag=f"xnp{c}",
                               name=f"xnp{c}_{s}") for c in range(NC_D)]
                for c in range(NC_D):
                    nc.vector.memset(xnp[c][:], 0.0)
                    xnp3 = xnp[c][:].rearrange("p (a b) -> p a b", a=PADW)
                    t13 = S["tm1"][c][:].rearrange("p (a b) -> p a b", a=GH)
                    r13 = S["rstd1"][:].rearrange("p (a b) -> p a b", a=GH)
                    if ln_trivial:
                        eng = nc.gpsimd if "gp_xnp" in opts else nc.vector
                        eng.tensor_tensor(
                            xnp3[:, 1:GH + 1, 1:GW + 1], t13[:, :, :],
                            r13[:, :, :], op=Alu.mult)
                    else:
                        nc.vector.scalar_tensor_tensor(
                            xnp3[:, 1:GH + 1, 1:GW + 1], t13[:, :, :],
                            gm1_t[:, c:c + 1], r13[:, :, :],
                            op0=Alu.mult, op1=Alu.mult)
                        nc.vector.tensor_scalar(
                            xnp3[:, 1:GH + 1, 1:GW + 1],
                            xnp3[:, 1:GH + 1, 1:GW + 1],
                            bt1_t[:, c:c + 1], None, op0=Alu.add)
                S["xnp"] = xnp

            def emit_conv(s):
                S = st[s]
                # fp8 conv output: chunks 0,1 as a DoubleRow pair block
                # [128, (i=2, c=1024)], chunk 2 flat for the plain fp8 matmul.
                xs8 = sb.tile([128, 2 * L], f8, tag="xs8", name=f"xs8_{s}")
                xs2 = sb.tile([128, L], f8, tag="xs2", name=f"xs2_{s}")
                if "conv" in ablate:
                    nc.vector.memset(xs8[:], 0.01)
                    nc.vector.memset(xs2[:], 0.01)
                for c in range(NC_D if "conv" not in ablate else 0):
                    xp3 = S["xnp"][c][:].rearrange("p (a b) -> p a b", a=PADW)
                    cv = ps.tile([128, L], f32, tag="bk", name=f"cv{c}_{s}")
                    for t in range(9):
                        ky, kx = divmod(t, 3)
                        for h in range(2):
                            rhs = xp3[:, ky + 16 * h: ky + 16 * h + 16,
                                      kx: kx + GW]
                            nc.tensor.matmul(cv[:, HS[h]], cd_t[t][c][:], rhs,
                                             start=(t == 0), stop=(t == 8))
                    dst = (xs8[:, c * L:(c + 1) * L] if c < 2 else xs2[:])
                    nc.scalar.activation(dst, cv[:], Act.Identity,
                                         bias=dwcb_t[:, c:c + 1])
                S["xs8"], S["xs2"] = xs8, xs2

            def emit_gru_dir(s, d):
                S = st[s]
                xs8, xs2 = S["xs8"], S["xs2"]
                x8r = xs8[:].rearrange("p (i c) -> p i c", i=2)
                wp_t = g1wp_t if d == 0 else g2wp_t
                w2_t = g1w2_t if d == 0 else g2w2_t
                wpr = wp_t[:].rearrange("p (i c) -> p i c", i=2)
                go_t = g1o_t if d == 0 else g2o_t
                tagp = "hh" if d == 0 else "yh"
                hh = [sb.tile([128, L], f16, tag=f"{tagp}{j}",
                              name=f"hh{j}_{s}{d}") for j in range(NC_H)]
                for j in range(NC_H):
                    Hp = ps.tile([128, L], f32, tag="bk", name=f"Hp{j}_{s}{d}")
                    Gp = ps.tile([128, L], f32, tag="bk", name=f"Gp{j}_{s}{d}")
                    for dst, j0 in ((Hp, j), (Gp, j + NC_H)):
                        cs = slice(j0 * 128, (j0 + 1) * 128)
                        for h in range(2):
                            hsl = slice(h * 512, (h + 1) * 512)
                            nc.tensor.matmul(
                                dst[:, HS[h]], wpr[:, :, cs], x8r[:, :, hsl],
                                start=True, stop=False,
                                perf_mode=PM.DoubleRow)
                        for h in range(2):
                            hsl = slice(h * 512, (h + 1) * 512)
                            nc.tensor.matmul(
                                dst[:, HS[h]], w2_t[:, cs], xs2[:, hsl],
                                start=False, stop=True)
                    if "ew" in ablate:
                        nc.vector.memset(hh[j][:], 0.01)
                        continue
                    z = sb.tile([128, L], f16, tag="z", name=f"z{s}{d}{j}",
                                bufs=2)
                    sh = sb.tile([128, L], f16, tag="sh", name=f"sh{s}{d}{j}",
                                 bufs=2)
                    nc.scalar.activation(z[:], Gp[:], Act.Sigmoid)
                    nc.scalar.activation(sh[:], Hp[:], Act.Sigmoid)
                    # g(H) = relu(H) + min(sigmoid(H), 0.5) == max(H + 0.5,
                    # sigmoid(H)): one STT replaces relu + min-add.
                    g = sb.tile([128, L], f16, tag="g", name=f"g{s}{d}{j}",
                                bufs=2)
                    nc.vector.scalar_tensor_tensor(
                        g[:], Hp[:], 0.5, sh[:], op0=Alu.add, op1=Alu.max)
                    b = sb.tile([128, L], f16, tag="b", name=f"b{s}{d}{j}",
                                bufs=2)
                    a = sb.tile([128, L], f16, tag="a", name=f"a{s}{d}{j}",
                                bufs=2)
                    if "gp_ab" in opts:
                        nc.gpsimd.tensor_tensor(b[:], z[:], g[:], op=Alu.mult)
                        nc.gpsimd.tensor_scalar(a[:], z[:], -1.0, 1.0,
                                                op0=Alu.mult, op1=Alu.add)
                    elif "gp_a" in opts:
                        nc.vector.tensor_tensor(b[:], z[:], g[:], op=Alu.mult)
                        nc.gpsimd.tensor_scalar(a[:], z[:], -1.0, 1.0,
                                                op0=Alu.mult, op1=Alu.add)
                    else:
                        nc.vector.tensor_tensor(b[:], z[:], g[:], op=Alu.mult)
                        nc.vector.tensor_scalar(a[:], z[:], -1.0, 1.0,
                                                op0=Alu.mult, op1=Alu.add)
                    if "scan" in ablate:
                        nc.vector.tensor_copy(hh[j][:], b[:])
                    elif d == 0:
                        nc.vector.tensor_tensor_scan(
                            hh[j][:], a[:], b[:], 0.0,
                            op0=Alu.mult, op1=Alu.add)
                    else:
                        nc.vector.tensor_tensor_scan(
                            hh[j][:, ::-1], a[:, ::-1], b[:, ::-1], 0.0,
                            op0=Alu.mult, op1=Alu.add)
                S[f"hh{d}"] = hh

            def emit_outproj(s, d):
                S = st[s]
                go_t = g1o_t if d == 0 else g2o_t
                hh = S[f"hh{d}"]
                if d == 0:
                    dst = [sb.tile([128, L], f16, tag=f"y0_{c}",
                                   name=f"y0_{c}_{s}") for c in range(NC_D)]
                    res = S["xt"]
                    S["y0"] = dst
                else:
                    dst = [sb.tile([128, L], f16, tag=f"yt{c}",
                                   name=f"yt{c}_{s}") for c in range(NC_D)]
                    res = S["y0"]
                    S["yt"] = dst
                for m in range(NC_D):
                    q = ps.tile([128, L], f32, tag="bk", name=f"xq{m}_{s}{d}")
                    for j in range(NC_H):
                        wj = go_t[j][:, m * 128:(m + 1) * 128]
                        for h in HS:
                            nc.tensor.matmul(
                                q[:, h], wj, hh[j][:, h],
                                start=(j == 0), stop=(j == NC_H - 1))
                    nc.vector.tensor_tensor(dst[m][:], q[:], res[m][:],
                                            op=Alu.add)

            def emit_ln2(s):
                S = st[s]
                S["tm2"], S["rstd2"] = layer_norm_stats(S["yt"], "b")
                yn = [sb.tile([128, L], f16, tag=f"yn{c}", name=f"yn{c}_{s}")
                      for c in range(NC_D)]
                for c in range(NC_D):
                    if ln_trivial:
                        nc.vector.tensor_tensor(yn[c][:], S["tm2"][c][:],
                                                S["rstd2"][:], op=Alu.mult)
                    else:
                        nc.vector.scalar_tensor_tensor(
                            yn[c][:], S["tm2"][c][:], gm2_t[:, c:c + 1],
                            S["rstd2"][:], op0=Alu.mult, op1=Alu.mult)
                        nc.vector.tensor_scalar(
                            yn[c][:], yn[c][:], bt2_t[:, c:c + 1], None,
                            op0=Alu.add)
                S["yn"] = yn

            def emit_mlp(s):
                S = st[s]
                yn = S["yn"]
                yh8 = [sb.tile([128, 2 * L], f8, tag=f"yh8_{u}",
                               name=f"yh8_{u}_{s}") for u in range(NC_M // 2)]
                for j in range(NC_M):
                    pph = ps.tile([128, L], f32, tag="bk", name=f"pp{j}_{s}")
                    for k in range(NC_D):
                        wj = p1w_t[k][:, j * 128:(j + 1) * 128]
                        for h in HS:
                            nc.tensor.matmul(
                                pph[:, h], wj, yn[k][:, h],
                                start=(k == 0), stop=(k == NC_D - 1))
                    u, i = divmod(j, 2)
                    nc.scalar.activation(yh8[u][:, i * L:(i + 1) * L], pph[:],
                                         Act.Gelu, bias=pb1_t[:, j:j + 1])
                yo = [sb.tile([128, L], f16, tag=f"y0_{c}", name=f"yo{c}_{s}")
                      for c in range(NC_D)]
                for m in range(NC_D):
                    q = ps.tile([128, L], f32, tag="bk", name=f"oq{m}_{s}")
                    for u in range(NC_M // 2):
                        w8 = p2w8_t[u][:].rearrange("p (i m) -> p i m", i=2)
                        y8 = yh8[u][:].rearrange("p (i c) -> p i c", i=2)
                        for h in range(2):
                            hsl = slice(h * 512, (h + 1) * 512)
                            nc.tensor.matmul(
                                q[:, HS[h]], w8[:, :, m * 128:(m + 1) * 128],
                                y8[:, :, hsl],
                                start=(u == 0), stop=(u == NC_M // 2 - 1),
                                perf_mode=PM.DoubleRow)
                    nc.vector.scalar_tensor_tensor(
                        yo[m][:], q[:], pb2_t[:, m:m + 1], S["yt"][m][:],
                        op0=Alu.add, op1=Alu.add)
                for c in range(NC_D):
                    nc.sync.dma_start(y_d.ap()[s, c * 128:(c + 1) * 128, :],
                                      yo[c][:])

            # Software pipeline: LN1 runs one sample ahead (its long ACT/DVE
            # chain resolves under MLP/GRU work), conv(s) uses last
            # iteration's xnp and covers the LN2(s-1)->yn chain before
            # MLP(s-1), whose matmuls in turn cover LN1(s+1)'s chain.
            emit_ln1(0)
            for s in range(ns):
                if s + 1 < ns:
                    emit_ln1(s + 1)
                emit_conv(s)
                if s >= 1:
                    emit_mlp(s - 1)
                emit_gru_dir(s, 0)
                emit_gru_dir(s, 1)   # hg(d1) fills PE while d0 scans run
                emit_outproj(s, 0)
                emit_outproj(s, 1)
                emit_ln2(s)
            emit_mlp(ns - 1)

    nc.compile()
    return nc


_NC_CACHE = {}


def _get_nc(ns=NS, num_devices=N_CORES, ln_trivial=True, pb2_zero=True):
    key = (ns, num_devices, ln_trivial, pb2_zero)
    if key not in _NC_CACHE:
        _NC_CACHE[key] = build_nc(ns, num_devices, ln_trivial=ln_trivial,
                                  pb2_zero=pb2_zero)
    return _NC_CACHE[key]


def make_weight_maps(gamma1, beta1, dwc_w, dwc_b, gru1_w, gru1_out,
                     gru2_w, gru2_out, gamma2, beta2, p1_w, p1_b, p2_w, p2_b):
    f = np.float32
    np8 = mybir.dt.np(f8)
    dwc = np.asarray(dwc_w, f).reshape(DIM, 9)
    cdiag = np.zeros((9, NC_D, 128, 128), np.float16)
    for t in range(9):
        for c in range(NC_D):
            np.einsum('ii->i', cdiag[t, c])[:] = dwc[c * 128:(c + 1) * 128, t]
    col = lambda v: np.ascontiguousarray(
        np.asarray(v, f).reshape(NC_D if len(np.asarray(v).reshape(-1)) == DIM
                                 else NC_M, 128).T)

    def gru_w8(w):
        w = np.asarray(w, f) * WSC
        wp = np.stack([w[0:128], w[128:256]], axis=1)       # [128, 2, 2*DI]
        return (np.ascontiguousarray(wp).astype(np8),
                np.ascontiguousarray(w[256:384]).astype(np8))

    g1wp, g1w2 = gru_w8(gru1_w)
    g2wp, g2w2 = gru_w8(gru2_w)
    p2 = np.asarray(p2_w, f) * WSC                          # [1536, 384]
    p2w8 = np.stack([np.stack([p2[(2 * u) * 128:(2 * u + 1) * 128],
                               p2[(2 * u + 1) * 128:(2 * u + 2) * 128]],
                              axis=1)
                     for u in range(NC_M // 2)])            # [6, 128, 2, 384]
    return dict(
        g1wp=g1wp, g1w2=g1w2, g2wp=g2wp, g2w2=g2w2,
        g1o=np.asarray(gru1_out, f).astype(np.float16),
        g2o=np.asarray(gru2_out, f).astype(np.float16),
        p1w=np.asarray(p1_w, f).astype(np.float16),
        p2w8=np.ascontiguousarray(p2w8).astype(np8),
        cdiag=cdiag,
        pb1=col(p1_b), pb2=col(p2_b), dwcb=col(dwc_b),
        gm1=col(gamma1), bt1=col(beta1), gm2=col(gamma2), bt2=col(beta2),
    )


def kernel(x, gamma1, beta1, dwc_w, dwc_b, gru1_w, gru1_out, gru2_w, gru2_out,
           gamma2, beta2, p1_w, p1_b, p2_w, p2_b, h, w):
    x = np.asarray(x, np.float32)
    n = x.shape[0]
    ln_trivial = (np.allclose(np.asarray(gamma1), 1.0)
                  and np.allclose(np.asarray(beta1), 0.0)
                  and np.allclose(np.asarray(gamma2), 1.0)
                  and np.allclose(np.asarray(beta2), 0.0))
    pb2_zero = bool(np.allclose(np.asarray(p2_b), 0.0))
    nc = _get_nc(ln_trivial=ln_trivial, pb2_zero=pb2_zero)
    xT = np.ascontiguousarray(x.transpose(0, 2, 1)).astype(np.float16)
    wmap = make_weight_maps(gamma1, beta1, dwc_w, dwc_b, gru1_w, gru1_out,
                            gru2_w, gru2_out, gamma2, beta2, p1_w, p1_b,
                            p2_w, p2_b)
    in_maps = []
    for i in range(N_CORES):
        m = dict(wmap)
        m["xT"] = xT[i * NS:(i + 1) * NS]
        in_maps.append(m)
    res = run_bass_kernel_spmd(nc, in_maps, list(range(N_CORES)))
    yT = np.concatenate([res.results[i]["y"] for i in range(N_CORES)], axis=0)
    return np.ascontiguousarray(yT.transpose(0, 2, 1)).astype(np.float32)
